# revision 44
# baseline (speedup 1.0000x reference)
"""MoE top-2 (2 experts) FFN kernel for TRN2, 8 NeuronCores.

Problem (hardcoded):
  x:   (8192, 2048) f32 tokens
  two expert FFNs: d_model=2048 -> d_ff=8192 (gelu exact) -> 2048
  out[i] = w0[i] * FFN0(x[i]) + w1[i] * FFN1(x[i])
  where w_e[i] = sum of top2_weight[i, s] over slots s with (top2_exp_id[i,s] % 2) == e

Strategy:
  - Host: fold top-2 gating into per-token scalars w0/w1; transpose x;
    gather each expert's active tokens (those with w_e > 0, ~75% of
    tokens) into a padded capacity of 784 per core -> 18.75% fewer FLOPs
    than dense. Dense fallback if a gather overflows capacity.
  - Data-parallel over tokens: each of 8 cores takes 1024 tokens.
  - bf16 weights + activations (fp32 PSUM accumulate, fp32 gelu/y):
    halves HBM traffic and enables FWL so LDWEIGHTS fully hides under
    the 1-cycle/row matmul stream; l2 err ~5e-3 vs the 2e-2 gate.
  - On-core: activations kept transposed ([d_model|d_ff on partitions] x
    [tokens on free dim]) so both matmul layers contract along partitions
    with weights in their natural HBM layout. W1 is host-packed into
    per-(k,chunk) contiguous 128KB strips for single-descriptor DMAs.
  - Both experts' gathered xT stay SBUF-resident (6.3MB bf16); expert 1's
    tiles prefetch during expert 0 compute -> no stall at the switch.
  - d_ff processed in chunks of 512; layer-2 partials accumulated into an
    SBUF-resident y so each weight byte is streamed exactly once.
  - Software-pipelined emission: PE order L1(0),L1(1),L2(0),L1(2),L2(1)...
    so gelu/gate (ACT+DVE) of chunk i overlaps L1(i+1) matmuls.
"""

import os

import ml_dtypes
import numpy as np

import concourse.bass as bass
import concourse.mybir as mybir
import concourse.tile as tile
from concourse import bacc
from concourse import bass_utils

BF16NP = ml_dtypes.bfloat16


def _ensure_ntff_hook():
    """This image's `antenv` lacks `axon_hooks`, so boot-time NTFF hook
    install degrades silently and trace=True captures nothing. Register a
    shim module and install the ctypes-driven hook (same as trn_boot)."""
    import sys
    import types

    if "antenv.axon_hooks" in sys.modules:
        return
    mod = types.ModuleType("antenv.axon_hooks")
    mod._hook = None

    def set_axon_ntff_profile_hook(h):
        mod._hook = h

    def get_axon_ntff_profile_hook():
        return mod._hook

    mod.set_axon_ntff_profile_hook = set_axon_ntff_profile_hook
    mod.get_axon_ntff_profile_hook = get_axon_ntff_profile_hook
    sys.modules["antenv.axon_hooks"] = mod
    try:
        from trn_agent_boot.trn_boot import _ntff_profile_via_ctypes

        hook = _ntff_profile_via_ctypes("/opt/axon/libaxon_pjrt.so")
        if hook is not None:
            mod._hook = hook
    except Exception:
        pass


P = 128
D_MODEL = 2048
D_FF = 8192
N_LOCAL = 8192
N_CORES = 8
TOKC = N_LOCAL // N_CORES      # 1024 tokens per core
CAP = 768                      # per-expert gathered-token capacity per core
                               # (seed-0 routing peaks at 767/765 per core;
                               # dense fallback covers anything bigger)
KM = D_MODEL // P              # 16 contraction tiles for layer 1
CHUNK = 512                    # d_ff chunk held in PSUM per pass
FC = CHUNK // P                # 4 d_ff tiles per chunk
NCHUNK = D_FF // CHUNK         # 16
M2 = D_MODEL // P              # 16 output d_model tiles

F32 = mybir.dt.float32
F32R = mybir.dt.float32r
BF16 = mybir.dt.bfloat16
GELU = mybir.ActivationFunctionType.Gelu


def _blocks(total):
    """Moving-dim blocks: each <= 512 (PSUM bank limit for f32 output).
    Equal blocks keep per-matmul overhead uniform."""
    n = (total + 511) // 512
    base = total // n
    out = []
    off = 0
    for i in range(n):
        hs = base + (1 if i < total - base * n else 0)
        out.append((off, hs))
        off += hs
    assert off == total and all(hs <= 512 for _, hs in out)
    return out


def _build_sparse(nc):
    """Per-expert gathered tokens (CAP per core); expert passes run
    back-to-back; both experts' xT tiles are SBUF-resident."""
    HS = _blocks(CAP)
    # xg host-packed partition-major: [p, k, t] -> per-partition lines of
    # (k1-k0)*CAP*2B per group DMA
    xg = [
        nc.dram_tensor(f"xg{e}", (P, KM, CAP), BF16, kind="ExternalInput").ap()
        for e in range(2)
    ]
    # w1 host-packed quad-strips: [q, c, p, kk, f] -> a [P, 4, CHUNK] quad
    # is one contiguous 512KB block with 4KB per-partition lines
    w1 = [
        nc.dram_tensor(f"w1_{e}", (KM // 4, NCHUNK, P, 4, CHUNK), BF16,
                       kind="ExternalInput").ap()
        for e in range(2)
    ]
    w2 = [
        nc.dram_tensor(f"w2_{e}", (D_FF, D_MODEL), BF16, kind="ExternalInput").ap()
        for e in range(2)
    ]
    b1t = [
        nc.dram_tensor(f"b1t_{e}", (P, D_FF // P), F32, kind="ExternalInput").ap()
        for e in range(2)
    ]
    b2t = [
        nc.dram_tensor(f"b2t_{e}", (P, M2), F32, kind="ExternalInput").ap()
        for e in range(2)
    ]
    wgg = [
        nc.dram_tensor(f"wgg{e}", (P, CAP), F32, kind="ExternalInput").ap()
        for e in range(2)
    ]
    # yt partition-major: [p, m, t]
    yt = [
        nc.dram_tensor(f"yt{e}", (P, M2, CAP), BF16, kind="ExternalOutput").ap()
        for e in range(2)
    ]

    with tile.TileContext(nc) as tc:
        with (
            tc.tile_pool(name="const", bufs=1) as const_pool,
            tc.tile_pool(name="w1s", bufs=8) as w1_pool,
            tc.tile_pool(name="w2s", bufs=6) as w2_pool,
            tc.tile_pool(name="ht", bufs=8) as ht_pool,
            tc.tile_pool(name="ps", bufs=8, space="PSUM") as psum_pool,
        ):
            # Both experts' xT k-tiles live in SBUF simultaneously (bf16).
            xt_sb = [
                const_pool.tile([P, KM, CAP], BF16, tag=f"xt{e}", name=f"xt_sb{e}")
                for e in range(2)
            ]
            y_sb = const_pool.tile([P, M2, CAP], F32, tag="y", name="y_sb")
            # final-chunk accumulate lands here in bf16 -> half-size stores
            yb_sb = const_pool.tile([P, M2, CAP], BF16, tag="yb", name="yb_sb")
            wgg_sb = [
                const_pool.tile([P, CAP], F32, tag=f"wgg{e}", name=f"wgg{e}_sb")
                for e in range(2)
            ]
            b1t_sb = [
                const_pool.tile([P, D_FF // P], F32, tag=f"b1t{e}", name=f"b1t{e}_sb")
                for e in range(2)
            ]
            b2t_sb = [
                const_pool.tile([P, M2], F32, tag=f"b2t{e}", name=f"b2t{e}_sb")
                for e in range(2)
            ]

            xg3 = xg
            yt3 = yt

            # chunk schedule: (expert, d_ff tile start fi0, n tiles fc)
            NFI = D_FF // P  # 64
            chunks = [
                (e, c * FC, FC) for e in range(2) for c in range(NCHUNK)
            ]

            # xT k-tile groups: small up front so the first matmuls aren't
            # gated on a big transfer, fat later. Groups MUST be emitted at
            # or before the w1-quad loop that consumes their k range.
            XT_GROUPS = [(0, 1), (1, 2), (2, 4), (4, 8), (8, 12), (12, 16)]

            # warm-up: garbage matmuls while the startup DMAs are in
            # flight, so HAM un-throttles the PE (~3.4us of activity)
            # before the first real matmul
            dummy = const_pool.tile([P, 256], BF16, tag="dummy", name="dummy_sb")
            nc.gpsimd.memset(dummy[:], 0)
            # preload the gelu table (1.3us ACT_TABLE_LOAD) off the
            # critical path, before the first real gelu needs it
            nc.scalar.activation(dummy[:, 128:256], dummy[:, :128], GELU)
            dps = psum_pool.tile([P, 128], F32, tag="ps", name="dummy_ps")
            for _ in range(16):
                nc.tensor.matmul(dps[:], dummy[:, :P], dummy[:, :P], start=True,
                                 stop=True)

            def emit_aux(e):
                # tiny b1t/b2t on scalar (they release ring credits fast
                # and gelu needs b1t); fat wgg goes on sync separately
                nc.scalar.dma_start(b1t_sb[e][:], b1t[e][:])
                nc.scalar.dma_start(b2t_sb[e][:], b2t[e][:])

            def emit_l1(ci, e, fi0, fc, first=False):
                """PE: layer-1 matmuls for one (expert, chunk)."""
                cb, fo = fi0 // FC, (fi0 % FC) * P
                ncol = fc * P
                psums = [
                    [
                        psum_pool.tile(
                            [P, hs], F32, tag="ps", name=f"ps1_{e}_{fi0}_{f}_{h}"
                        )
                        for h, (off, hs) in enumerate(HS)
                    ]
                    for f in range(fc)
                ]
                for q in range(KM // 4):
                    # quad-strip w1 tile: 4 k-strips per DMA enqueue
                    w1s = w1_pool.tile(
                        [P, 4, CHUNK], BF16, tag="w1s", name=f"w1s_{e}_{fi0}_{q}"
                    )
                    if first and q == 0:
                        # sub-loads so the first matmuls gate on 64KB
                        for k0, k1 in ((0, 1), (1, 2), (2, 4)):
                            nc.sync.dma_start(
                                w1s[:, k0:k1, :ncol],
                                w1[e][q, cb, :, k0:k1, fo : fo + ncol],
                            )
                    else:
                        nc.sync.dma_start(
                            w1s[:, :, :ncol], w1[e][q, cb, :, :, fo : fo + ncol]
                        )
                    if first:
                        # aux first: tiny, needed by the first gelu; the
                        # ONLY scalar-queue DMAs before the first gelu
                        # (shared ring credits would block it otherwise)
                        if q == 0:
                            emit_aux(0)
                        # expert-0 xT: the two tiny head groups ride the
                        # scalar queue (their rings clear long before the
                        # first gelu), the fat rest interleave with the w1
                        # quads on sync in need-order
                        for gi in ({0: (0, 1, 2), 1: (3,), 2: (4,), 3: (5,)}[q]):
                            k0, k1 = XT_GROUPS[gi]
                            nc.sync.dma_start(
                                xt_sb[0][:, k0:k1, :], xg3[0][:, k0:k1, :]
                            )
                        if q == 3:
                            nc.sync.dma_start(wgg_sb[0][:], wgg[0][:])
                    if ci == 1 and q == 0:
                        emit_aux(1)
                        nc.sync.dma_start(wgg_sb[1][:], wgg[1][:])
                    if 3 <= ci <= 6 and q == 0:
                        # prefetch expert-1 xT (one 4-tile group per chunk)
                        # well before the expert switch, on the scalar queue
                        k0 = 4 * (ci - 3)
                        nc.scalar.dma_start(
                            xt_sb[1][:, k0 : k0 + 4, :], xg3[1][:, k0 : k0 + 4, :]
                        )
                    for kk in range(4):
                        k = 4 * q + kk
                        for f in range(fc):
                            for h, (off, hs) in enumerate(HS):
                                nc.tensor.matmul(
                                    psums[f][h][:],
                                    w1s[:, kk, f * P : (f + 1) * P],
                                    xt_sb[e][:, k, off : off + hs],
                                    start=(k == 0),
                                    stop=(k == KM - 1),
                                )
                return psums

            def emit_act(e, fi0, fc, psums, first=False):
                """ACT+DVE: gelu(+b1), gate scale. Also W2 strip loads
                (scalar queue, enqueued BEFORE the gelu ops so they don't
                wait behind them in the FIFO — except the very first chunk,
                where the startup DMA crunch would exhaust ring credits and
                block the first gelu, stalling the next chunk's PSUM
                release), and (on each expert's first chunk) the gated b2
                y-init."""
                def emit_w2():
                    w2s = []
                    for f in range(fc):
                        w2f = w2_pool.tile(
                            [P, D_MODEL], BF16, tag="w2s",
                            name=f"w2s_{e}_{fi0}_{f}"
                        )
                        row = (fi0 + f) * P
                        nc.scalar.dma_start(w2f[:], w2[e][row : row + P, :])
                        w2s.append(w2f)
                    return w2s

                w2s = None if first else emit_w2()
                if fi0 == 0:
                    for m in range(M2):
                        nc.vector.tensor_scalar_mul(
                            y_sb[:, m, :], wgg_sb[e][:], b2t_sb[e][:, m : m + 1]
                        )
                hts = []
                for f in range(fc):
                    ht = ht_pool.tile(
                        [P, CAP], BF16, tag="ht", name=f"ht_{e}_{fi0}_{f}"
                    )
                    col = fi0 + f
                    for h, (off, hs) in enumerate(HS):
                        nc.scalar.activation(
                            ht[:, off : off + hs],
                            psums[f][h][:],
                            GELU,
                            bias=b1t_sb[e][:, col : col + 1],
                        )
                    nc.vector.tensor_mul(ht[:], ht[:], wgg_sb[e][:])
                    hts.append(ht)
                if w2s is None:
                    w2s = emit_w2()
                return hts, w2s

            def emit_l2(e, fi0, fc, hts, w2s):
                """PE: layer-2 matmuls; DVE: accumulate into y; the last
                chunk's accumulate writes bf16 and stores half the bytes."""
                last = fi0 + fc == NFI
                for m in range(M2):
                    for h, (off, hs) in enumerate(HS):
                        ps = psum_pool.tile(
                            [P, hs], F32, tag="ps", name=f"ps2_{e}_{fi0}_{m}_{h}"
                        )
                        for f in range(fc):
                            nc.tensor.matmul(
                                ps[:],
                                w2s[f][:, m * P : (m + 1) * P],
                                hts[f][:, off : off + hs],
                                start=(f == 0),
                                stop=(f == fc - 1),
                            )
                        ysl = y_sb[:, m, off : off + hs]
                        if last:
                            nc.vector.tensor_add(
                                yb_sb[:, m, off : off + hs], ysl, ps[:]
                            )
                        else:
                            nc.vector.tensor_add(ysl, ysl, ps[:])
                    if last:
                        # split the final stores across both HW-DGE queues
                        eng = nc.sync if m % 2 == 0 else nc.scalar
                        eng.dma_start(yt3[e][:, m, :], yb_sb[:, m, :])

            psums_cur = emit_l1(0, *chunks[0], first=True)
            for i, (e, fi0, fc) in enumerate(chunks):
                hts, w2s = emit_act(e, fi0, fc, psums_cur, first=(i == 0))
                if i + 1 < len(chunks):
                    psums_cur = emit_l1(i + 1, *chunks[i + 1])
                emit_l2(e, fi0, fc, hts, w2s)

    nc.compile()
    return nc


def _build_dense(nc):
    """Dense fallback: both experts over all tokens, gate-weighted."""
    HS = [(0, 512), (512, 512)]
    xt = nc.dram_tensor("xt", (D_MODEL, TOKC), F32R, kind="ExternalInput").ap()
    w1 = [
        nc.dram_tensor(f"w1_{e}", (D_MODEL, D_FF), F32R, kind="ExternalInput").ap()
        for e in range(2)
    ]
    w2 = [
        nc.dram_tensor(f"w2_{e}", (D_FF, D_MODEL), F32R, kind="ExternalInput").ap()
        for e in range(2)
    ]
    b1t = [
        nc.dram_tensor(f"b1t_{e}", (P, D_FF // P), F32, kind="ExternalInput").ap()
        for e in range(2)
    ]
    b2t = [
        nc.dram_tensor(f"b2t_{e}", (P, M2), F32, kind="ExternalInput").ap()
        for e in range(2)
    ]
    wg = [
        nc.dram_tensor(f"wg{e}", (P, TOKC), F32, kind="ExternalInput").ap()
        for e in range(2)
    ]
    yt = nc.dram_tensor("yt", (D_MODEL, TOKC), F32, kind="ExternalOutput").ap()

    with tile.TileContext(nc) as tc:
        with (
            tc.tile_pool(name="const", bufs=1) as const_pool,
            tc.tile_pool(name="w1s", bufs=5) as w1_pool,
            tc.tile_pool(name="w2s", bufs=5) as w2_pool,
            tc.tile_pool(name="ht", bufs=5) as ht_pool,
            tc.tile_pool(name="ps", bufs=8, space="PSUM") as psum_pool,
        ):
            xt_sb = const_pool.tile([P, KM, TOKC], F32R, tag="xt", name="xt_sb")
            y_sb = const_pool.tile([P, M2, TOKC], F32, tag="y", name="y_sb")
            wg_sb = [
                const_pool.tile([P, TOKC], F32, tag=f"wg{e}", name=f"wg{e}_sb")
                for e in range(2)
            ]
            b1t_sb = [
                const_pool.tile([P, D_FF // P], F32, tag=f"b1t{e}", name=f"b1t{e}_sb")
                for e in range(2)
            ]
            b2t_sb = [
                const_pool.tile([P, M2], F32, tag=f"b2t{e}", name=f"b2t{e}_sb")
                for e in range(2)
            ]

            xt3 = xt.rearrange("(ko p) t -> p ko t", p=P)
            pairs = [(e, c) for e in range(2) for c in range(NCHUNK)]

            def emit_l1(e, c, first=False):
                psums = [
                    [
                        psum_pool.tile(
                            [P, hs], F32, tag="ps", name=f"ps1_{e}_{c}_{f}_{h}"
                        )
                        for h, (off, hs) in enumerate(HS)
                    ]
                    for f in range(FC)
                ]
                for k in range(KM):
                    if first:
                        nc.sync.dma_start(xt_sb[:, k, :], xt3[:, k, :])
                        if k == 0:
                            for ee in range(2):
                                nc.sync.dma_start(wg_sb[ee][:], wg[ee][:])
                                nc.sync.dma_start(b1t_sb[ee][:], b1t[ee][:])
                                nc.sync.dma_start(b2t_sb[ee][:], b2t[ee][:])
                    w1s = w1_pool.tile(
                        [P, CHUNK], F32R, tag="w1s", name=f"w1s_{e}_{c}_{k}"
                    )
                    nc.sync.dma_start(
                        w1s[:],
                        w1[e][k * P : (k + 1) * P, c * CHUNK : (c + 1) * CHUNK],
                    )
                    for f in range(FC):
                        for h, (off, hs) in enumerate(HS):
                            nc.tensor.matmul(
                                psums[f][h][:],
                                w1s[:, f * P : (f + 1) * P],
                                xt_sb[:, k, off : off + hs],
                                start=(k == 0),
                                stop=(k == KM - 1),
                            )
                return psums

            def emit_act(e, c, psums):
                hts = []
                for f in range(FC):
                    ht = ht_pool.tile(
                        [P, TOKC], F32R, tag="ht", name=f"ht_{e}_{c}_{f}"
                    )
                    col = c * FC + f
                    for h, (off, hs) in enumerate(HS):
                        nc.scalar.activation(
                            ht[:, off : off + hs],
                            psums[f][h][:],
                            GELU,
                            bias=b1t_sb[e][:, col : col + 1],
                        )
                    nc.vector.tensor_mul(ht[:], ht[:], wg_sb[e][:])
                    hts.append(ht)
                w2s = []
                for f in range(FC):
                    w2f = w2_pool.tile(
                        [P, D_MODEL], F32R, tag="w2s", name=f"w2s_{e}_{c}_{f}"
                    )
                    row = (c * FC + f) * P
                    nc.sync.dma_start(w2f[:], w2[e][row : row + P, :])
                    w2s.append(w2f)
                return hts, w2s

            def emit_l2(e, c, hts, w2s):
                for m in range(M2):
                    for h, (off, hs) in enumerate(HS):
                        ps = psum_pool.tile(
                            [P, hs], F32, tag="ps", name=f"ps2_{e}_{c}_{m}_{h}"
                        )
                        for f in range(FC):
                            nc.tensor.matmul(
                                ps[:],
                                w2s[f][:, m * P : (m + 1) * P],
                                hts[f][:, off : off + hs],
                                start=(f == 0),
                                stop=(f == FC - 1),
                            )
                        ysl = y_sb[:, m, off : off + hs]
                        nc.vector.tensor_add(ysl, ysl, ps[:])

            psums_cur = emit_l1(*pairs[0], first=True)

            for m in range(M2):
                nc.vector.tensor_scalar_mul(
                    y_sb[:, m, :], wg_sb[0][:], b2t_sb[0][:, m : m + 1]
                )
                t = ht_pool.tile([P, TOKC], F32, tag="ht", name="ytmp")
                nc.vector.tensor_scalar_mul(
                    t[:], wg_sb[1][:], b2t_sb[1][:, m : m + 1]
                )
                nc.vector.tensor_add(y_sb[:, m, :], y_sb[:, m, :], t[:])

            for i, (e, c) in enumerate(pairs):
                hts, w2s = emit_act(e, c, psums_cur)
                if i + 1 < len(pairs):
                    psums_cur = emit_l1(*pairs[i + 1])
                emit_l2(e, c, hts, w2s)

            yt3 = yt.rearrange("(mo p) t -> p mo t", p=P)
            for m in range(M2):
                nc.sync.dma_start(yt3[:, m, :], y_sb[:, m, :])

    nc.compile()
    return nc


_CACHED = {}


def _get_nc(kind):
    if kind not in _CACHED:
        nc = bacc.Bacc(
            "TRN2",
            target_bir_lowering=False,
            debug=False,
            num_devices=N_CORES,
        )
        _CACHED[kind] = (_build_sparse if kind == "sparse" else _build_dense)(nc)
    return _CACHED[kind]


def _run(nc, in_maps):
    trace = bool(int(os.environ.get("KERNEL_TRACE", "0")))
    if trace:
        _ensure_ntff_hook()
    res = bass_utils.run_bass_kernel_spmd(
        nc, in_maps, core_ids=list(range(N_CORES)), trace=trace
    )
    if trace:
        kernel.last_exec_time_ns = res.exec_time_ns
        kernel.last_results = res
    return res


def kernel(**inputs):
    x = np.asarray(inputs["x_local"], dtype=np.float32)          # (8192, 2048)
    ids = np.asarray(inputs["top2_exp_id"])                       # (8192, 2)
    tw = np.asarray(inputs["top2_weight"], dtype=np.float32)      # (8192, 2)

    sel = (ids % 2).astype(np.float32)
    wge = [
        (tw * (1.0 - sel)).sum(axis=1).astype(np.float32),        # expert-0 gate
        (tw * sel).sum(axis=1).astype(np.float32),                # expert-1 gate
    ]

    xtb = np.ascontiguousarray(x.T.astype(BF16NP))                # (2048, 8192) bf16

    sparse_shared = {}
    for e in range(2):
        w1b = np.asarray(inputs[f"W1_{e}"], dtype=np.float32).astype(BF16NP)
        # pack into quad-strip blocks [q, c, p, kk, f]: one [P,4,CHUNK]
        # quad per DMA with 4KB contiguous per-partition lines
        sparse_shared[f"w1_{e}"] = np.ascontiguousarray(
            w1b.reshape(KM // 4, 4, P, NCHUNK, CHUNK).transpose(0, 3, 2, 1, 4)
        )
        sparse_shared[f"w2_{e}"] = np.ascontiguousarray(
            np.asarray(inputs[f"W2_{e}"], dtype=np.float32).astype(BF16NP)
        )
        sparse_shared[f"b1t_{e}"] = np.ascontiguousarray(
            np.asarray(inputs[f"b1_{e}"], dtype=np.float32).reshape(D_FF // P, P).T
        )
        sparse_shared[f"b2t_{e}"] = np.ascontiguousarray(
            np.asarray(inputs[f"b2_{e}"], dtype=np.float32).reshape(M2, P).T
        )

    # Globally-balanced gathers: each expert's active set (~75% of all
    # tokens) is split evenly across the 8 cores, so per-core load is
    # |S_e|/8 +- 1 regardless of which core a token "belongs" to.
    glocs = [np.flatnonzero(wge[e] > 0) for e in range(2)]
    overflow = any(len(g) > CAP * N_CORES for g in glocs)

    if not overflow:
        splits = [np.array_split(glocs[e], N_CORES) for e in range(2)]
        in_maps = []
        for c in range(N_CORES):
            m = dict(sparse_shared)
            for e in range(2):
                loc = splits[e][c]
                cnt = len(loc)
                xgc = np.zeros((D_MODEL, CAP), BF16NP)
                xgc[:, :cnt] = xtb[:, loc]
                # partition-major [p, k, t]
                m[f"xg{e}"] = np.ascontiguousarray(
                    xgc.reshape(KM, P, CAP).transpose(1, 0, 2)
                )
                wggc = np.zeros((CAP,), np.float32)
                wggc[:cnt] = wge[e][loc]
                m[f"wgg{e}"] = np.ascontiguousarray(
                    np.broadcast_to(wggc, (P, CAP))
                )
            in_maps.append(m)

        print("PATH: sparse", flush=True)
        res = _run(_get_nc("sparse"), in_maps)

        y = np.zeros((N_LOCAL, D_MODEL), np.float32)
        for c in range(N_CORES):
            for e in range(2):
                loc = splits[e][c]
                cnt = len(loc)
                r = res.results[c][f"yt{e}"]  # (P, M2, CAP) bf16
                yd = r.transpose(1, 0, 2).reshape(D_MODEL, CAP)
                y[loc] += yd.T[:cnt].astype(np.float32)
        return y

    # dense fallback (vanishingly rare: a gather exceeded capacity)
    xt = np.ascontiguousarray(x.T)
    dense_shared = {}
    for e in range(2):
        dense_shared[f"w1_{e}"] = np.ascontiguousarray(
            np.asarray(inputs[f"W1_{e}"], dtype=np.float32)
        )
        dense_shared[f"w2_{e}"] = np.ascontiguousarray(
            np.asarray(inputs[f"W2_{e}"], dtype=np.float32)
        )
        dense_shared[f"b1t_{e}"] = sparse_shared[f"b1t_{e}"]
        dense_shared[f"b2t_{e}"] = sparse_shared[f"b2t_{e}"]
    in_maps = []
    for c in range(N_CORES):
        tok = slice(c * TOKC, (c + 1) * TOKC)
        m = dict(dense_shared)
        m["xt"] = np.ascontiguousarray(xt[:, tok])
        for e in range(2):
            m[f"wg{e}"] = np.ascontiguousarray(
                np.broadcast_to(wge[e][tok], (P, TOKC)).astype(np.float32)
            )
        in_maps.append(m)
    print("PATH: dense", flush=True)
    res = _run(_get_nc("dense"), in_maps)
    ytc = np.concatenate([r["yt"] for r in res.results], axis=1)  # (2048, 8192)
    return np.ascontiguousarray(ytc.T)


# revision 50
# speedup vs baseline: 1.7295x; 1.7295x over previous
"""MoE top-2 (2 experts) FFN kernel for TRN2, 8 NeuronCores.

Problem (hardcoded):
  x:   (8192, 2048) f32 tokens
  two expert FFNs: d_model=2048 -> d_ff=8192 (gelu exact) -> 2048
  out[i] = w0[i] * FFN0(x[i]) + w1[i] * FFN1(x[i])
  where w_e[i] = sum of top2_weight[i, s] over slots s with (top2_exp_id[i,s] % 2) == e

Strategy:
  - Host: fold top-2 gating into per-token scalars w0/w1; transpose x;
    gather each expert's active tokens (those with w_e > 0, ~75% of
    tokens) into a padded capacity of 784 per core -> 18.75% fewer FLOPs
    than dense. Dense fallback if a gather overflows capacity.
  - Data-parallel over tokens: each of 8 cores takes 1024 tokens.
  - bf16 weights + activations (fp32 PSUM accumulate, fp32 gelu/y):
    halves HBM traffic and enables FWL so LDWEIGHTS fully hides under
    the 1-cycle/row matmul stream; l2 err ~5e-3 vs the 2e-2 gate.
  - On-core: activations kept transposed ([d_model|d_ff on partitions] x
    [tokens on free dim]) so both matmul layers contract along partitions
    with weights in their natural HBM layout. W1 is host-packed into
    per-(k,chunk) contiguous 128KB strips for single-descriptor DMAs.
  - Both experts' gathered xT stay SBUF-resident (6.3MB bf16); expert 1's
    tiles prefetch during expert 0 compute -> no stall at the switch.
  - d_ff processed in chunks of 512; layer-2 partials accumulated into an
    SBUF-resident y so each weight byte is streamed exactly once.
  - Software-pipelined emission: PE order L1(0),L1(1),L2(0),L1(2),L2(1)...
    so gelu/gate (ACT+DVE) of chunk i overlaps L1(i+1) matmuls.
"""

import os

import ml_dtypes
import numpy as np

import concourse.bass as bass
import concourse.mybir as mybir
import concourse.tile as tile
from concourse import bacc
from concourse import bass_utils

BF16NP = ml_dtypes.bfloat16


def _ensure_ntff_hook():
    """This image's `antenv` lacks `axon_hooks`, so boot-time NTFF hook
    install degrades silently and trace=True captures nothing. Register a
    shim module and install the ctypes-driven hook (same as trn_boot)."""
    import sys
    import types

    if "antenv.axon_hooks" in sys.modules:
        return
    mod = types.ModuleType("antenv.axon_hooks")
    mod._hook = None

    def set_axon_ntff_profile_hook(h):
        mod._hook = h

    def get_axon_ntff_profile_hook():
        return mod._hook

    mod.set_axon_ntff_profile_hook = set_axon_ntff_profile_hook
    mod.get_axon_ntff_profile_hook = get_axon_ntff_profile_hook
    sys.modules["antenv.axon_hooks"] = mod
    try:
        from trn_agent_boot.trn_boot import _ntff_profile_via_ctypes

        hook = _ntff_profile_via_ctypes("/opt/axon/libaxon_pjrt.so")
        if hook is not None:
            mod._hook = hook
    except Exception:
        pass


P = 128
D_MODEL = 2048
D_FF = 8192
N_LOCAL = 8192
N_CORES = 8
TOKC = N_LOCAL // N_CORES      # 1024 tokens per core
CAP_MAX = 832                  # per-expert gathered-token capacity limit
                               # (SBUF budget); dense fallback above this.
                               # actual CAP is chosen per call from the
                               # observed routing counts.
KM = D_MODEL // P              # 16 contraction tiles for layer 1
CHUNK = 512                    # d_ff chunk held in PSUM per pass
FC = CHUNK // P                # 4 d_ff tiles per chunk
NCHUNK = D_FF // CHUNK         # 16
M2 = D_MODEL // P              # 16 output d_model tiles

F32 = mybir.dt.float32
F32R = mybir.dt.float32r
BF16 = mybir.dt.bfloat16
GELU = mybir.ActivationFunctionType.Gelu


def _blocks(total):
    """Moving-dim blocks: each <= 512 (PSUM bank limit for f32 output).
    Equal blocks keep per-matmul overhead uniform."""
    n = (total + 511) // 512
    base = total // n
    out = []
    off = 0
    for i in range(n):
        hs = base + (1 if i < total - base * n else 0)
        out.append((off, hs))
        off += hs
    assert off == total and all(hs <= 512 for _, hs in out)
    return out


def _build_sparse(nc, CAP):
    """Per-expert gathered tokens (CAP per core); expert passes run
    back-to-back; both experts' xT tiles are SBUF-resident."""
    HS = _blocks(CAP)
    # xg host-packed partition-major: [p, k, t] -> per-partition lines of
    # (k1-k0)*CAP*2B per group DMA
    xg = [
        nc.dram_tensor(f"xg{e}", (P, KM, CAP), BF16, kind="ExternalInput").ap()
        for e in range(2)
    ]
    # w1 host-packed quad-strips: [q, c, p, kk, f] -> a [P, 4, CHUNK] quad
    # is one contiguous 512KB block with 4KB per-partition lines
    w1 = [
        nc.dram_tensor(f"w1_{e}", (KM // 4, NCHUNK, P, 4, CHUNK), BF16,
                       kind="ExternalInput").ap()
        for e in range(2)
    ]
    w2 = [
        nc.dram_tensor(f"w2_{e}", (D_FF, D_MODEL), BF16, kind="ExternalInput").ap()
        for e in range(2)
    ]
    b1t = [
        nc.dram_tensor(f"b1t_{e}", (P, D_FF // P), F32, kind="ExternalInput").ap()
        for e in range(2)
    ]
    b2t = [
        nc.dram_tensor(f"b2t_{e}", (P, M2), F32, kind="ExternalInput").ap()
        for e in range(2)
    ]
    wgg = [
        nc.dram_tensor(f"wgg{e}", (P, CAP), F32, kind="ExternalInput").ap()
        for e in range(2)
    ]
    # yt partition-major: [p, m, t]
    yt = [
        nc.dram_tensor(f"yt{e}", (P, M2, CAP), BF16, kind="ExternalOutput").ap()
        for e in range(2)
    ]

    with tile.TileContext(nc) as tc:
        with (
            tc.tile_pool(name="const", bufs=1) as const_pool,
            tc.tile_pool(name="w1s", bufs=8) as w1_pool,
            tc.tile_pool(name="w2s", bufs=6) as w2_pool,
            tc.tile_pool(name="ht", bufs=8) as ht_pool,
            tc.tile_pool(name="ps", bufs=8, space="PSUM") as psum_pool,
        ):
            # Both experts' xT k-tiles live in SBUF simultaneously (bf16).
            xt_sb = [
                const_pool.tile([P, KM, CAP], BF16, tag=f"xt{e}", name=f"xt_sb{e}")
                for e in range(2)
            ]
            y_sb = const_pool.tile([P, M2, CAP], F32, tag="y", name="y_sb")
            # final-chunk accumulate lands here in bf16 -> half-size stores
            yb_sb = const_pool.tile([P, M2, CAP], BF16, tag="yb", name="yb_sb")
            wgg_sb = [
                const_pool.tile([P, CAP], F32, tag=f"wgg{e}", name=f"wgg{e}_sb")
                for e in range(2)
            ]
            b1t_sb = [
                const_pool.tile([P, D_FF // P], F32, tag=f"b1t{e}", name=f"b1t{e}_sb")
                for e in range(2)
            ]
            b2t_sb = [
                const_pool.tile([P, M2], F32, tag=f"b2t{e}", name=f"b2t{e}_sb")
                for e in range(2)
            ]

            xg3 = xg
            yt3 = yt

            # chunk schedule: (expert, d_ff tile start fi0, n tiles fc)
            NFI = D_FF // P  # 64
            chunks = [
                (e, c * FC, FC) for e in range(2) for c in range(NCHUNK)
            ]

            # xT k-tile groups: small up front so the first matmuls aren't
            # gated on a big transfer, fat later. Groups MUST be emitted at
            # or before the w1-quad loop that consumes their k range.
            XT_GROUPS = [(0, 1), (1, 2), (2, 4), (4, 8), (8, 12), (12, 16)]

            # warm-up: garbage matmuls while the startup DMAs are in
            # flight, so HAM un-throttles the PE (~3.4us of activity)
            # before the first real matmul
            dummy = const_pool.tile([P, 256], BF16, tag="dummy", name="dummy_sb")
            nc.gpsimd.memset(dummy[:], 0)
            # preload the gelu table (1.3us ACT_TABLE_LOAD) off the
            # critical path, before the first real gelu needs it
            nc.scalar.activation(dummy[:, 128:256], dummy[:, :128], GELU)
            dps = psum_pool.tile([P, 128], F32, tag="ps", name="dummy_ps")
            for _ in range(16):
                nc.tensor.matmul(dps[:], dummy[:, :P], dummy[:, :P], start=True,
                                 stop=True)

            def emit_aux(e):
                # tiny b1t/b2t on scalar (they release ring credits fast
                # and gelu needs b1t); fat wgg goes on sync separately
                nc.scalar.dma_start(b1t_sb[e][:], b1t[e][:])
                nc.scalar.dma_start(b2t_sb[e][:], b2t[e][:])

            def emit_l1(ci, e, fi0, fc, first=False):
                """PE: layer-1 matmuls for one (expert, chunk)."""
                cb, fo = fi0 // FC, (fi0 % FC) * P
                ncol = fc * P
                psums = [
                    [
                        psum_pool.tile(
                            [P, hs], F32, tag="ps", name=f"ps1_{e}_{fi0}_{f}_{h}"
                        )
                        for h, (off, hs) in enumerate(HS)
                    ]
                    for f in range(fc)
                ]
                for q in range(KM // 4):
                    # quad-strip w1 tile: 4 k-strips per DMA enqueue
                    w1s = w1_pool.tile(
                        [P, 4, CHUNK], BF16, tag="w1s", name=f"w1s_{e}_{fi0}_{q}"
                    )
                    if first and q == 0:
                        # sub-loads so the first matmuls gate on 64KB
                        for k0, k1 in ((0, 1), (1, 2), (2, 4)):
                            nc.sync.dma_start(
                                w1s[:, k0:k1, :ncol],
                                w1[e][q, cb, :, k0:k1, fo : fo + ncol],
                            )
                    else:
                        nc.sync.dma_start(
                            w1s[:, :, :ncol], w1[e][q, cb, :, :, fo : fo + ncol]
                        )
                    if first:
                        # aux first: tiny, needed by the first gelu; the
                        # ONLY scalar-queue DMAs before the first gelu
                        # (shared ring credits would block it otherwise)
                        if q == 0:
                            emit_aux(0)
                        # expert-0 xT: the two tiny head groups ride the
                        # scalar queue (their rings clear long before the
                        # first gelu), the fat rest interleave with the w1
                        # quads on sync in need-order
                        for gi in ({0: (0, 1, 2), 1: (3,), 2: (4,), 3: (5,)}[q]):
                            k0, k1 = XT_GROUPS[gi]
                            nc.sync.dma_start(
                                xt_sb[0][:, k0:k1, :], xg3[0][:, k0:k1, :]
                            )
                        if q == 3:
                            nc.sync.dma_start(wgg_sb[0][:], wgg[0][:])
                    if ci == 1 and q == 0:
                        emit_aux(1)
                        nc.sync.dma_start(wgg_sb[1][:], wgg[1][:])
                    if 3 <= ci <= 6 and q == 0:
                        # prefetch expert-1 xT (one 4-tile group per chunk)
                        # well before the expert switch, on the scalar queue
                        k0 = 4 * (ci - 3)
                        nc.scalar.dma_start(
                            xt_sb[1][:, k0 : k0 + 4, :], xg3[1][:, k0 : k0 + 4, :]
                        )
                    for kk in range(4):
                        k = 4 * q + kk
                        for f in range(fc):
                            for h, (off, hs) in enumerate(HS):
                                nc.tensor.matmul(
                                    psums[f][h][:],
                                    w1s[:, kk, f * P : (f + 1) * P],
                                    xt_sb[e][:, k, off : off + hs],
                                    start=(k == 0),
                                    stop=(k == KM - 1),
                                )
                return psums

            def emit_act(e, fi0, fc, psums, first=False):
                """ACT+DVE: gelu(+b1), gate scale. Also W2 strip loads
                (scalar queue, enqueued BEFORE the gelu ops so they don't
                wait behind them in the FIFO — except the very first chunk,
                where the startup DMA crunch would exhaust ring credits and
                block the first gelu, stalling the next chunk's PSUM
                release), and (on each expert's first chunk) the gated b2
                y-init."""
                def emit_w2():
                    w2s = []
                    for f in range(fc):
                        w2f = w2_pool.tile(
                            [P, D_MODEL], BF16, tag="w2s",
                            name=f"w2s_{e}_{fi0}_{f}"
                        )
                        row = (fi0 + f) * P
                        nc.scalar.dma_start(w2f[:], w2[e][row : row + P, :])
                        w2s.append(w2f)
                    return w2s

                w2s = None if first else emit_w2()
                if fi0 == 0:
                    for m in range(M2):
                        nc.vector.tensor_scalar_mul(
                            y_sb[:, m, :], wgg_sb[e][:], b2t_sb[e][:, m : m + 1]
                        )
                hts = []
                for f in range(fc):
                    ht = ht_pool.tile(
                        [P, CAP], BF16, tag="ht", name=f"ht_{e}_{fi0}_{f}"
                    )
                    col = fi0 + f
                    for h, (off, hs) in enumerate(HS):
                        nc.scalar.activation(
                            ht[:, off : off + hs],
                            psums[f][h][:],
                            GELU,
                            bias=b1t_sb[e][:, col : col + 1],
                        )
                    nc.vector.tensor_mul(ht[:], ht[:], wgg_sb[e][:])
                    hts.append(ht)
                if w2s is None:
                    w2s = emit_w2()
                return hts, w2s

            def emit_l2(e, fi0, fc, hts, w2s):
                """PE: layer-2 matmuls; DVE: accumulate into y; the last
                chunk's accumulate writes bf16 and stores half the bytes."""
                last = fi0 + fc == NFI
                for m in range(M2):
                    for h, (off, hs) in enumerate(HS):
                        ps = psum_pool.tile(
                            [P, hs], F32, tag="ps", name=f"ps2_{e}_{fi0}_{m}_{h}"
                        )
                        for f in range(fc):
                            nc.tensor.matmul(
                                ps[:],
                                w2s[f][:, m * P : (m + 1) * P],
                                hts[f][:, off : off + hs],
                                start=(f == 0),
                                stop=(f == fc - 1),
                            )
                        ysl = y_sb[:, m, off : off + hs]
                        if last:
                            nc.vector.tensor_add(
                                yb_sb[:, m, off : off + hs], ysl, ps[:]
                            )
                        else:
                            nc.vector.tensor_add(ysl, ysl, ps[:])
                    if last:
                        # split the final stores across both HW-DGE queues
                        eng = nc.sync if m % 2 == 0 else nc.scalar
                        eng.dma_start(yt3[e][:, m, :], yb_sb[:, m, :])

            psums_cur = emit_l1(0, *chunks[0], first=True)
            for i, (e, fi0, fc) in enumerate(chunks):
                hts, w2s = emit_act(e, fi0, fc, psums_cur, first=(i == 0))
                if i + 1 < len(chunks):
                    psums_cur = emit_l1(i + 1, *chunks[i + 1])
                emit_l2(e, fi0, fc, hts, w2s)

    nc.compile()
    return nc


def _build_dense(nc):
    """Dense fallback: both experts over all tokens, gate-weighted."""
    HS = [(0, 512), (512, 512)]
    xt = nc.dram_tensor("xt", (D_MODEL, TOKC), F32R, kind="ExternalInput").ap()
    w1 = [
        nc.dram_tensor(f"w1_{e}", (D_MODEL, D_FF), F32R, kind="ExternalInput").ap()
        for e in range(2)
    ]
    w2 = [
        nc.dram_tensor(f"w2_{e}", (D_FF, D_MODEL), F32R, kind="ExternalInput").ap()
        for e in range(2)
    ]
    b1t = [
        nc.dram_tensor(f"b1t_{e}", (P, D_FF // P), F32, kind="ExternalInput").ap()
        for e in range(2)
    ]
    b2t = [
        nc.dram_tensor(f"b2t_{e}", (P, M2), F32, kind="ExternalInput").ap()
        for e in range(2)
    ]
    wg = [
        nc.dram_tensor(f"wg{e}", (P, TOKC), F32, kind="ExternalInput").ap()
        for e in range(2)
    ]
    yt = nc.dram_tensor("yt", (D_MODEL, TOKC), F32, kind="ExternalOutput").ap()

    with tile.TileContext(nc) as tc:
        with (
            tc.tile_pool(name="const", bufs=1) as const_pool,
            tc.tile_pool(name="w1s", bufs=5) as w1_pool,
            tc.tile_pool(name="w2s", bufs=5) as w2_pool,
            tc.tile_pool(name="ht", bufs=5) as ht_pool,
            tc.tile_pool(name="ps", bufs=8, space="PSUM") as psum_pool,
        ):
            xt_sb = const_pool.tile([P, KM, TOKC], F32R, tag="xt", name="xt_sb")
            y_sb = const_pool.tile([P, M2, TOKC], F32, tag="y", name="y_sb")
            wg_sb = [
                const_pool.tile([P, TOKC], F32, tag=f"wg{e}", name=f"wg{e}_sb")
                for e in range(2)
            ]
            b1t_sb = [
                const_pool.tile([P, D_FF // P], F32, tag=f"b1t{e}", name=f"b1t{e}_sb")
                for e in range(2)
            ]
            b2t_sb = [
                const_pool.tile([P, M2], F32, tag=f"b2t{e}", name=f"b2t{e}_sb")
                for e in range(2)
            ]

            xt3 = xt.rearrange("(ko p) t -> p ko t", p=P)
            pairs = [(e, c) for e in range(2) for c in range(NCHUNK)]

            def emit_l1(e, c, first=False):
                psums = [
                    [
                        psum_pool.tile(
                            [P, hs], F32, tag="ps", name=f"ps1_{e}_{c}_{f}_{h}"
                        )
                        for h, (off, hs) in enumerate(HS)
                    ]
                    for f in range(FC)
                ]
                for k in range(KM):
                    if first:
                        nc.sync.dma_start(xt_sb[:, k, :], xt3[:, k, :])
                        if k == 0:
                            for ee in range(2):
                                nc.sync.dma_start(wg_sb[ee][:], wg[ee][:])
                                nc.sync.dma_start(b1t_sb[ee][:], b1t[ee][:])
                                nc.sync.dma_start(b2t_sb[ee][:], b2t[ee][:])
                    w1s = w1_pool.tile(
                        [P, CHUNK], F32R, tag="w1s", name=f"w1s_{e}_{c}_{k}"
                    )
                    nc.sync.dma_start(
                        w1s[:],
                        w1[e][k * P : (k + 1) * P, c * CHUNK : (c + 1) * CHUNK],
                    )
                    for f in range(FC):
                        for h, (off, hs) in enumerate(HS):
                            nc.tensor.matmul(
                                psums[f][h][:],
                                w1s[:, f * P : (f + 1) * P],
                                xt_sb[:, k, off : off + hs],
                                start=(k == 0),
                                stop=(k == KM - 1),
                            )
                return psums

            def emit_act(e, c, psums):
                hts = []
                for f in range(FC):
                    ht = ht_pool.tile(
                        [P, TOKC], F32R, tag="ht", name=f"ht_{e}_{c}_{f}"
                    )
                    col = c * FC + f
                    for h, (off, hs) in enumerate(HS):
                        nc.scalar.activation(
                            ht[:, off : off + hs],
                            psums[f][h][:],
                            GELU,
                            bias=b1t_sb[e][:, col : col + 1],
                        )
                    nc.vector.tensor_mul(ht[:], ht[:], wg_sb[e][:])
                    hts.append(ht)
                w2s = []
                for f in range(FC):
                    w2f = w2_pool.tile(
                        [P, D_MODEL], F32R, tag="w2s", name=f"w2s_{e}_{c}_{f}"
                    )
                    row = (c * FC + f) * P
                    nc.sync.dma_start(w2f[:], w2[e][row : row + P, :])
                    w2s.append(w2f)
                return hts, w2s

            def emit_l2(e, c, hts, w2s):
                for m in range(M2):
                    for h, (off, hs) in enumerate(HS):
                        ps = psum_pool.tile(
                            [P, hs], F32, tag="ps", name=f"ps2_{e}_{c}_{m}_{h}"
                        )
                        for f in range(FC):
                            nc.tensor.matmul(
                                ps[:],
                                w2s[f][:, m * P : (m + 1) * P],
                                hts[f][:, off : off + hs],
                                start=(f == 0),
                                stop=(f == FC - 1),
                            )
                        ysl = y_sb[:, m, off : off + hs]
                        nc.vector.tensor_add(ysl, ysl, ps[:])

            psums_cur = emit_l1(*pairs[0], first=True)

            for m in range(M2):
                nc.vector.tensor_scalar_mul(
                    y_sb[:, m, :], wg_sb[0][:], b2t_sb[0][:, m : m + 1]
                )
                t = ht_pool.tile([P, TOKC], F32, tag="ht", name="ytmp")
                nc.vector.tensor_scalar_mul(
                    t[:], wg_sb[1][:], b2t_sb[1][:, m : m + 1]
                )
                nc.vector.tensor_add(y_sb[:, m, :], y_sb[:, m, :], t[:])

            for i, (e, c) in enumerate(pairs):
                hts, w2s = emit_act(e, c, psums_cur)
                if i + 1 < len(pairs):
                    psums_cur = emit_l1(*pairs[i + 1])
                emit_l2(e, c, hts, w2s)

            yt3 = yt.rearrange("(mo p) t -> p mo t", p=P)
            for m in range(M2):
                nc.sync.dma_start(yt3[:, m, :], y_sb[:, m, :])

    nc.compile()
    return nc


_CACHED = {}


def _get_nc(kind, cap=None):
    key = (kind, cap)
    if key not in _CACHED:
        nc = bacc.Bacc(
            "TRN2",
            target_bir_lowering=False,
            debug=False,
            num_devices=N_CORES,
        )
        if kind == "sparse":
            _CACHED[key] = _build_sparse(nc, cap)
        else:
            _CACHED[key] = _build_dense(nc)
    return _CACHED[key]


def _run(nc, in_maps):
    trace = bool(int(os.environ.get("KERNEL_TRACE", "0")))
    if trace:
        _ensure_ntff_hook()
    res = bass_utils.run_bass_kernel_spmd(
        nc, in_maps, core_ids=list(range(N_CORES)), trace=trace
    )
    if trace:
        kernel.last_exec_time_ns = res.exec_time_ns
        kernel.last_results = res
    return res


def kernel(**inputs):
    x = np.asarray(inputs["x_local"], dtype=np.float32)          # (8192, 2048)
    ids = np.asarray(inputs["top2_exp_id"])                       # (8192, 2)
    tw = np.asarray(inputs["top2_weight"], dtype=np.float32)      # (8192, 2)

    sel = (ids % 2).astype(np.float32)
    wge = [
        (tw * (1.0 - sel)).sum(axis=1).astype(np.float32),        # expert-0 gate
        (tw * sel).sum(axis=1).astype(np.float32),                # expert-1 gate
    ]

    xtb = np.ascontiguousarray(x.T.astype(BF16NP))                # (2048, 8192) bf16

    sparse_shared = {}
    for e in range(2):
        w1b = np.asarray(inputs[f"W1_{e}"], dtype=np.float32).astype(BF16NP)
        # pack into quad-strip blocks [q, c, p, kk, f]: one [P,4,CHUNK]
        # quad per DMA with 4KB contiguous per-partition lines
        sparse_shared[f"w1_{e}"] = np.ascontiguousarray(
            w1b.reshape(KM // 4, 4, P, NCHUNK, CHUNK).transpose(0, 3, 2, 1, 4)
        )
        sparse_shared[f"w2_{e}"] = np.ascontiguousarray(
            np.asarray(inputs[f"W2_{e}"], dtype=np.float32).astype(BF16NP)
        )
        sparse_shared[f"b1t_{e}"] = np.ascontiguousarray(
            np.asarray(inputs[f"b1_{e}"], dtype=np.float32).reshape(D_FF // P, P).T
        )
        sparse_shared[f"b2t_{e}"] = np.ascontiguousarray(
            np.asarray(inputs[f"b2_{e}"], dtype=np.float32).reshape(M2, P).T
        )

    # Globally-balanced gathers: each expert's active set (~75% of all
    # tokens) is split evenly across the 8 cores, so per-core load is
    # |S_e|/8 +- 1 regardless of which core a token "belongs" to.
    # CAP is sized to the observed routing (rounded up to a multiple of
    # 16 so the compiled kernel is reused across calls with similar
    # counts); compile time is per-CAP one-time, off the HW clock.
    glocs = [np.flatnonzero(wge[e] > 0) for e in range(2)]
    maxper = max(-(-len(g) // N_CORES) for g in glocs)
    CAP = max(512, ((maxper + 15) // 16) * 16)
    overflow = CAP > CAP_MAX

    if not overflow:
        splits = [np.array_split(glocs[e], N_CORES) for e in range(2)]
        in_maps = []
        for c in range(N_CORES):
            m = dict(sparse_shared)
            for e in range(2):
                loc = splits[e][c]
                cnt = len(loc)
                xgc = np.zeros((D_MODEL, CAP), BF16NP)
                xgc[:, :cnt] = xtb[:, loc]
                # partition-major [p, k, t]
                m[f"xg{e}"] = np.ascontiguousarray(
                    xgc.reshape(KM, P, CAP).transpose(1, 0, 2)
                )
                wggc = np.zeros((CAP,), np.float32)
                wggc[:cnt] = wge[e][loc]
                m[f"wgg{e}"] = np.ascontiguousarray(
                    np.broadcast_to(wggc, (P, CAP))
                )
            in_maps.append(m)

        res = _run(_get_nc("sparse", CAP), in_maps)

        y = np.zeros((N_LOCAL, D_MODEL), np.float32)
        for c in range(N_CORES):
            for e in range(2):
                loc = splits[e][c]
                cnt = len(loc)
                r = res.results[c][f"yt{e}"]  # (P, M2, CAP) bf16
                yd = r.transpose(1, 0, 2).reshape(D_MODEL, CAP)
                y[loc] += yd.T[:cnt].astype(np.float32)
        return y

    # dense fallback (vanishingly rare: a gather exceeded capacity)
    xt = np.ascontiguousarray(x.T)
    dense_shared = {}
    for e in range(2):
        dense_shared[f"w1_{e}"] = np.ascontiguousarray(
            np.asarray(inputs[f"W1_{e}"], dtype=np.float32)
        )
        dense_shared[f"w2_{e}"] = np.ascontiguousarray(
            np.asarray(inputs[f"W2_{e}"], dtype=np.float32)
        )
        dense_shared[f"b1t_{e}"] = sparse_shared[f"b1t_{e}"]
        dense_shared[f"b2t_{e}"] = sparse_shared[f"b2t_{e}"]
    in_maps = []
    for c in range(N_CORES):
        tok = slice(c * TOKC, (c + 1) * TOKC)
        m = dict(dense_shared)
        m["xt"] = np.ascontiguousarray(xt[:, tok])
        for e in range(2):
            m[f"wg{e}"] = np.ascontiguousarray(
                np.broadcast_to(wge[e][tok], (P, TOKC)).astype(np.float32)
            )
        in_maps.append(m)
    res = _run(_get_nc("dense"), in_maps)
    ytc = np.concatenate([r["yt"] for r in res.results], axis=1)  # (2048, 8192)
    return np.ascontiguousarray(ytc.T)


# revision 51
# speedup vs baseline: 1.7481x; 1.0108x over previous
"""MoE top-2 (2 experts) FFN kernel for TRN2, 8 NeuronCores.

Problem (hardcoded):
  x:   (8192, 2048) f32 tokens
  two expert FFNs: d_model=2048 -> d_ff=8192 (gelu exact) -> 2048
  out[i] = w0[i] * FFN0(x[i]) + w1[i] * FFN1(x[i])
  where w_e[i] = sum of top2_weight[i, s] over slots s with (top2_exp_id[i,s] % 2) == e

Strategy:
  - Host: fold top-2 gating into per-token scalars w0/w1; transpose x;
    gather each expert's active tokens (those with w_e > 0, ~75% of
    tokens) into a padded capacity of 784 per core -> 18.75% fewer FLOPs
    than dense. Dense fallback if a gather overflows capacity.
  - Data-parallel over tokens: each of 8 cores takes 1024 tokens.
  - bf16 weights + activations (fp32 PSUM accumulate, fp32 gelu/y):
    halves HBM traffic and enables FWL so LDWEIGHTS fully hides under
    the 1-cycle/row matmul stream; l2 err ~5e-3 vs the 2e-2 gate.
  - On-core: activations kept transposed ([d_model|d_ff on partitions] x
    [tokens on free dim]) so both matmul layers contract along partitions
    with weights in their natural HBM layout. W1 is host-packed into
    per-(k,chunk) contiguous 128KB strips for single-descriptor DMAs.
  - Both experts' gathered xT stay SBUF-resident (6.3MB bf16); expert 1's
    tiles prefetch during expert 0 compute -> no stall at the switch.
  - d_ff processed in chunks of 512; layer-2 partials accumulated into an
    SBUF-resident y so each weight byte is streamed exactly once.
  - Software-pipelined emission: PE order L1(0),L1(1),L2(0),L1(2),L2(1)...
    so gelu/gate (ACT+DVE) of chunk i overlaps L1(i+1) matmuls.
"""

import os

import ml_dtypes
import numpy as np

import concourse.bass as bass
import concourse.mybir as mybir
import concourse.tile as tile
from concourse import bacc
from concourse import bass_utils

BF16NP = ml_dtypes.bfloat16


def _ensure_ntff_hook():
    """This image's `antenv` lacks `axon_hooks`, so boot-time NTFF hook
    install degrades silently and trace=True captures nothing. Register a
    shim module and install the ctypes-driven hook (same as trn_boot)."""
    import sys
    import types

    if "antenv.axon_hooks" in sys.modules:
        return
    mod = types.ModuleType("antenv.axon_hooks")
    mod._hook = None

    def set_axon_ntff_profile_hook(h):
        mod._hook = h

    def get_axon_ntff_profile_hook():
        return mod._hook

    mod.set_axon_ntff_profile_hook = set_axon_ntff_profile_hook
    mod.get_axon_ntff_profile_hook = get_axon_ntff_profile_hook
    sys.modules["antenv.axon_hooks"] = mod
    try:
        from trn_agent_boot.trn_boot import _ntff_profile_via_ctypes

        hook = _ntff_profile_via_ctypes("/opt/axon/libaxon_pjrt.so")
        if hook is not None:
            mod._hook = hook
    except Exception:
        pass


P = 128
D_MODEL = 2048
D_FF = 8192
N_LOCAL = 8192
N_CORES = 8
TOKC = N_LOCAL // N_CORES      # 1024 tokens per core
CAP_MAX = 832                  # per-expert gathered-token capacity limit
                               # (SBUF budget); dense fallback above this.
                               # actual CAP is chosen per call from the
                               # observed routing counts.
KM = D_MODEL // P              # 16 contraction tiles for layer 1
CHUNK = 512                    # d_ff chunk held in PSUM per pass
FC = CHUNK // P                # 4 d_ff tiles per chunk
NCHUNK = D_FF // CHUNK         # 16
M2 = D_MODEL // P              # 16 output d_model tiles

F32 = mybir.dt.float32
F32R = mybir.dt.float32r
BF16 = mybir.dt.bfloat16
GELU = mybir.ActivationFunctionType.Gelu


def _blocks(total):
    """Moving-dim blocks: each <= 512 (PSUM bank limit for f32 output).
    Equal blocks keep per-matmul overhead uniform."""
    n = (total + 511) // 512
    base = total // n
    out = []
    off = 0
    for i in range(n):
        hs = base + (1 if i < total - base * n else 0)
        out.append((off, hs))
        off += hs
    assert off == total and all(hs <= 512 for _, hs in out)
    return out


def _build_sparse(nc, CAP):
    """Per-expert gathered tokens (CAP per core); expert passes run
    back-to-back; both experts' xT tiles are SBUF-resident."""
    HS = _blocks(CAP)
    # xg host-packed partition-major: [p, k, t] -> per-partition lines of
    # (k1-k0)*CAP*2B per group DMA
    xg = [
        nc.dram_tensor(f"xg{e}", (P, KM, CAP), BF16, kind="ExternalInput").ap()
        for e in range(2)
    ]
    # w1 host-packed quad-strips: [q, c, p, kk, f] -> a [P, 4, CHUNK] quad
    # is one contiguous 512KB block with 4KB per-partition lines
    w1 = [
        nc.dram_tensor(f"w1_{e}", (KM // 4, NCHUNK, P, 4, CHUNK), BF16,
                       kind="ExternalInput").ap()
        for e in range(2)
    ]
    w2 = [
        nc.dram_tensor(f"w2_{e}", (D_FF, D_MODEL), BF16, kind="ExternalInput").ap()
        for e in range(2)
    ]
    b1t = [
        nc.dram_tensor(f"b1t_{e}", (P, D_FF // P), F32, kind="ExternalInput").ap()
        for e in range(2)
    ]
    b2t = [
        nc.dram_tensor(f"b2t_{e}", (P, M2), F32, kind="ExternalInput").ap()
        for e in range(2)
    ]
    wgg = [
        nc.dram_tensor(f"wgg{e}", (P, CAP), F32, kind="ExternalInput").ap()
        for e in range(2)
    ]
    # yt partition-major: [p, m, t]
    yt = [
        nc.dram_tensor(f"yt{e}", (P, M2, CAP), BF16, kind="ExternalOutput").ap()
        for e in range(2)
    ]

    with tile.TileContext(nc) as tc:
        with (
            tc.tile_pool(name="const", bufs=1) as const_pool,
            tc.tile_pool(name="w1s", bufs=8) as w1_pool,
            tc.tile_pool(name="w2s", bufs=6) as w2_pool,
            tc.tile_pool(name="ht", bufs=8) as ht_pool,
            tc.tile_pool(name="ps", bufs=8, space="PSUM") as psum_pool,
        ):
            # Both experts' xT k-tiles live in SBUF simultaneously (bf16).
            xt_sb = [
                const_pool.tile([P, KM, CAP], BF16, tag=f"xt{e}", name=f"xt_sb{e}")
                for e in range(2)
            ]
            y_sb = const_pool.tile([P, M2, CAP], F32, tag="y", name="y_sb")
            # final-chunk accumulate lands here in bf16 -> half-size stores
            yb_sb = const_pool.tile([P, M2, CAP], BF16, tag="yb", name="yb_sb")
            wgg_sb = [
                const_pool.tile([P, CAP], F32, tag=f"wgg{e}", name=f"wgg{e}_sb")
                for e in range(2)
            ]
            b1t_sb = [
                const_pool.tile([P, D_FF // P], F32, tag=f"b1t{e}", name=f"b1t{e}_sb")
                for e in range(2)
            ]
            b2t_sb = [
                const_pool.tile([P, M2], F32, tag=f"b2t{e}", name=f"b2t{e}_sb")
                for e in range(2)
            ]

            xg3 = xg
            yt3 = yt

            # chunk schedule: (expert, d_ff tile start fi0, n tiles fc)
            NFI = D_FF // P  # 64
            chunks = [
                (e, c * FC, FC) for e in range(2) for c in range(NCHUNK)
            ]

            # xT k-tile groups: small up front so the first matmuls aren't
            # gated on a big transfer, fat later. Groups MUST be emitted at
            # or before the w1-quad loop that consumes their k range.
            XT_GROUPS = [(0, 1), (1, 2), (2, 4), (4, 8), (8, 12), (12, 16)]

            # warm-up: garbage matmuls while the startup DMAs are in
            # flight, so HAM un-throttles the PE (~3.4us of activity)
            # before the first real matmul
            dummy = const_pool.tile([P, 256], BF16, tag="dummy", name="dummy_sb")
            nc.gpsimd.memset(dummy[:], 0)
            # preload the gelu table (1.3us ACT_TABLE_LOAD) off the
            # critical path, before the first real gelu needs it
            nc.scalar.activation(dummy[:, 128:256], dummy[:, :128], GELU)
            dps = psum_pool.tile([P, 128], F32, tag="ps", name="dummy_ps")
            for _ in range(16):
                nc.tensor.matmul(dps[:], dummy[:, :P], dummy[:, :P], start=True,
                                 stop=True)

            def emit_aux(e):
                # tiny b1t/b2t on scalar (they release ring credits fast
                # and gelu needs b1t); fat wgg goes on sync separately
                nc.scalar.dma_start(b1t_sb[e][:], b1t[e][:])
                nc.scalar.dma_start(b2t_sb[e][:], b2t[e][:])

            def emit_l1(ci, e, fi0, fc, first=False):
                """PE: layer-1 matmuls for one (expert, chunk)."""
                cb, fo = fi0 // FC, (fi0 % FC) * P
                ncol = fc * P
                psums = [
                    [
                        psum_pool.tile(
                            [P, hs], F32, tag="ps", name=f"ps1_{e}_{fi0}_{f}_{h}"
                        )
                        for h, (off, hs) in enumerate(HS)
                    ]
                    for f in range(fc)
                ]
                for q in range(KM // 4):
                    # quad-strip w1 tile: 4 k-strips per DMA enqueue
                    w1s = w1_pool.tile(
                        [P, 4, CHUNK], BF16, tag="w1s", name=f"w1s_{e}_{fi0}_{q}"
                    )
                    if first and q == 0:
                        # sub-loads so the first matmuls gate on 64KB
                        for k0, k1 in ((0, 1), (1, 2), (2, 4)):
                            nc.sync.dma_start(
                                w1s[:, k0:k1, :ncol],
                                w1[e][q, cb, :, k0:k1, fo : fo + ncol],
                            )
                    else:
                        nc.sync.dma_start(
                            w1s[:, :, :ncol], w1[e][q, cb, :, :, fo : fo + ncol]
                        )
                    if first:
                        # aux first: tiny, needed by the first gelu; the
                        # ONLY scalar-queue DMAs before the first gelu
                        # (shared ring credits would block it otherwise)
                        if q == 0:
                            emit_aux(0)
                        # expert-0 xT: the two tiny head groups ride the
                        # scalar queue (their rings clear long before the
                        # first gelu), the fat rest interleave with the w1
                        # quads on sync in need-order
                        for gi in ({0: (0, 1, 2), 1: (3,), 2: (4,), 3: (5,)}[q]):
                            k0, k1 = XT_GROUPS[gi]
                            nc.sync.dma_start(
                                xt_sb[0][:, k0:k1, :], xg3[0][:, k0:k1, :]
                            )
                        if q == 3:
                            nc.sync.dma_start(wgg_sb[0][:], wgg[0][:])
                    if ci == 1 and q == 0:
                        emit_aux(1)
                        nc.sync.dma_start(wgg_sb[1][:], wgg[1][:])
                    if 3 <= ci <= 6 and q == 0:
                        # prefetch expert-1 xT (one 4-tile group per chunk)
                        # well before the expert switch, on the scalar queue
                        k0 = 4 * (ci - 3)
                        nc.scalar.dma_start(
                            xt_sb[1][:, k0 : k0 + 4, :], xg3[1][:, k0 : k0 + 4, :]
                        )
                    for kk in range(4):
                        k = 4 * q + kk
                        for f in range(fc):
                            for h, (off, hs) in enumerate(HS):
                                nc.tensor.matmul(
                                    psums[f][h][:],
                                    w1s[:, kk, f * P : (f + 1) * P],
                                    xt_sb[e][:, k, off : off + hs],
                                    start=(k == 0),
                                    stop=(k == KM - 1),
                                )
                return psums

            def emit_act(e, fi0, fc, psums, first=False):
                """ACT+DVE: gelu(+b1), gate scale. Also W2 strip loads
                (scalar queue, enqueued BEFORE the gelu ops so they don't
                wait behind them in the FIFO — except the very first chunk,
                where the startup DMA crunch would exhaust ring credits and
                block the first gelu, stalling the next chunk's PSUM
                release), and (on each expert's first chunk) the gated b2
                y-init."""
                def emit_w2():
                    w2s = []
                    for f in range(fc):
                        w2f = w2_pool.tile(
                            [P, D_MODEL], BF16, tag="w2s",
                            name=f"w2s_{e}_{fi0}_{f}"
                        )
                        row = (fi0 + f) * P
                        nc.scalar.dma_start(w2f[:], w2[e][row : row + P, :])
                        w2s.append(w2f)
                    return w2s

                w2s = None if first else emit_w2()
                if fi0 == 0:
                    for m in range(M2):
                        nc.vector.tensor_scalar_mul(
                            y_sb[:, m, :], wgg_sb[e][:], b2t_sb[e][:, m : m + 1]
                        )
                hts = []
                for f in range(fc):
                    ht = ht_pool.tile(
                        [P, CAP], BF16, tag="ht", name=f"ht_{e}_{fi0}_{f}"
                    )
                    col = fi0 + f
                    for h, (off, hs) in enumerate(HS):
                        nc.scalar.activation(
                            ht[:, off : off + hs],
                            psums[f][h][:],
                            GELU,
                            bias=b1t_sb[e][:, col : col + 1],
                        )
                    nc.vector.tensor_mul(ht[:], ht[:], wgg_sb[e][:])
                    hts.append(ht)
                if w2s is None:
                    w2s = emit_w2()
                return hts, w2s

            def emit_l2(e, fi0, fc, hts, w2s):
                """PE: layer-2 matmuls; DVE: accumulate into y; the last
                chunk's accumulate writes bf16 and stores half the bytes."""
                last = fi0 + fc == NFI
                for m in range(M2):
                    for h, (off, hs) in enumerate(HS):
                        ps = psum_pool.tile(
                            [P, hs], F32, tag="ps", name=f"ps2_{e}_{fi0}_{m}_{h}"
                        )
                        for f in range(fc):
                            nc.tensor.matmul(
                                ps[:],
                                w2s[f][:, m * P : (m + 1) * P],
                                hts[f][:, off : off + hs],
                                start=(f == 0),
                                stop=(f == fc - 1),
                            )
                        ysl = y_sb[:, m, off : off + hs]
                        if last:
                            nc.vector.tensor_add(
                                yb_sb[:, m, off : off + hs], ysl, ps[:]
                            )
                        else:
                            nc.vector.tensor_add(ysl, ysl, ps[:])
                    if last:
                        # split the final stores across both HW-DGE queues
                        eng = nc.sync if m % 2 == 0 else nc.scalar
                        eng.dma_start(yt3[e][:, m, :], yb_sb[:, m, :])

            psums_cur = emit_l1(0, *chunks[0], first=True)
            for i, (e, fi0, fc) in enumerate(chunks):
                hts, w2s = emit_act(e, fi0, fc, psums_cur, first=(i == 0))
                if i + 1 < len(chunks):
                    psums_cur = emit_l1(i + 1, *chunks[i + 1])
                emit_l2(e, fi0, fc, hts, w2s)

    nc.compile()
    return nc


def _build_dense(nc):
    """Dense fallback: both experts over all tokens, gate-weighted."""
    HS = [(0, 512), (512, 512)]
    xt = nc.dram_tensor("xt", (D_MODEL, TOKC), F32R, kind="ExternalInput").ap()
    w1 = [
        nc.dram_tensor(f"w1_{e}", (D_MODEL, D_FF), F32R, kind="ExternalInput").ap()
        for e in range(2)
    ]
    w2 = [
        nc.dram_tensor(f"w2_{e}", (D_FF, D_MODEL), F32R, kind="ExternalInput").ap()
        for e in range(2)
    ]
    b1t = [
        nc.dram_tensor(f"b1t_{e}", (P, D_FF // P), F32, kind="ExternalInput").ap()
        for e in range(2)
    ]
    b2t = [
        nc.dram_tensor(f"b2t_{e}", (P, M2), F32, kind="ExternalInput").ap()
        for e in range(2)
    ]
    wg = [
        nc.dram_tensor(f"wg{e}", (P, TOKC), F32, kind="ExternalInput").ap()
        for e in range(2)
    ]
    yt = nc.dram_tensor("yt", (D_MODEL, TOKC), F32, kind="ExternalOutput").ap()

    with tile.TileContext(nc) as tc:
        with (
            tc.tile_pool(name="const", bufs=1) as const_pool,
            tc.tile_pool(name="w1s", bufs=5) as w1_pool,
            tc.tile_pool(name="w2s", bufs=5) as w2_pool,
            tc.tile_pool(name="ht", bufs=5) as ht_pool,
            tc.tile_pool(name="ps", bufs=8, space="PSUM") as psum_pool,
        ):
            xt_sb = const_pool.tile([P, KM, TOKC], F32R, tag="xt", name="xt_sb")
            y_sb = const_pool.tile([P, M2, TOKC], F32, tag="y", name="y_sb")
            wg_sb = [
                const_pool.tile([P, TOKC], F32, tag=f"wg{e}", name=f"wg{e}_sb")
                for e in range(2)
            ]
            b1t_sb = [
                const_pool.tile([P, D_FF // P], F32, tag=f"b1t{e}", name=f"b1t{e}_sb")
                for e in range(2)
            ]
            b2t_sb = [
                const_pool.tile([P, M2], F32, tag=f"b2t{e}", name=f"b2t{e}_sb")
                for e in range(2)
            ]

            xt3 = xt.rearrange("(ko p) t -> p ko t", p=P)
            pairs = [(e, c) for e in range(2) for c in range(NCHUNK)]

            def emit_l1(e, c, first=False):
                psums = [
                    [
                        psum_pool.tile(
                            [P, hs], F32, tag="ps", name=f"ps1_{e}_{c}_{f}_{h}"
                        )
                        for h, (off, hs) in enumerate(HS)
                    ]
                    for f in range(FC)
                ]
                for k in range(KM):
                    if first:
                        nc.sync.dma_start(xt_sb[:, k, :], xt3[:, k, :])
                        if k == 0:
                            for ee in range(2):
                                nc.sync.dma_start(wg_sb[ee][:], wg[ee][:])
                                nc.sync.dma_start(b1t_sb[ee][:], b1t[ee][:])
                                nc.sync.dma_start(b2t_sb[ee][:], b2t[ee][:])
                    w1s = w1_pool.tile(
                        [P, CHUNK], F32R, tag="w1s", name=f"w1s_{e}_{c}_{k}"
                    )
                    nc.sync.dma_start(
                        w1s[:],
                        w1[e][k * P : (k + 1) * P, c * CHUNK : (c + 1) * CHUNK],
                    )
                    for f in range(FC):
                        for h, (off, hs) in enumerate(HS):
                            nc.tensor.matmul(
                                psums[f][h][:],
                                w1s[:, f * P : (f + 1) * P],
                                xt_sb[:, k, off : off + hs],
                                start=(k == 0),
                                stop=(k == KM - 1),
                            )
                return psums

            def emit_act(e, c, psums):
                hts = []
                for f in range(FC):
                    ht = ht_pool.tile(
                        [P, TOKC], F32R, tag="ht", name=f"ht_{e}_{c}_{f}"
                    )
                    col = c * FC + f
                    for h, (off, hs) in enumerate(HS):
                        nc.scalar.activation(
                            ht[:, off : off + hs],
                            psums[f][h][:],
                            GELU,
                            bias=b1t_sb[e][:, col : col + 1],
                        )
                    nc.vector.tensor_mul(ht[:], ht[:], wg_sb[e][:])
                    hts.append(ht)
                w2s = []
                for f in range(FC):
                    w2f = w2_pool.tile(
                        [P, D_MODEL], F32R, tag="w2s", name=f"w2s_{e}_{c}_{f}"
                    )
                    row = (c * FC + f) * P
                    nc.sync.dma_start(w2f[:], w2[e][row : row + P, :])
                    w2s.append(w2f)
                return hts, w2s

            def emit_l2(e, c, hts, w2s):
                for m in range(M2):
                    for h, (off, hs) in enumerate(HS):
                        ps = psum_pool.tile(
                            [P, hs], F32, tag="ps", name=f"ps2_{e}_{c}_{m}_{h}"
                        )
                        for f in range(FC):
                            nc.tensor.matmul(
                                ps[:],
                                w2s[f][:, m * P : (m + 1) * P],
                                hts[f][:, off : off + hs],
                                start=(f == 0),
                                stop=(f == FC - 1),
                            )
                        ysl = y_sb[:, m, off : off + hs]
                        nc.vector.tensor_add(ysl, ysl, ps[:])

            psums_cur = emit_l1(*pairs[0], first=True)

            for m in range(M2):
                nc.vector.tensor_scalar_mul(
                    y_sb[:, m, :], wg_sb[0][:], b2t_sb[0][:, m : m + 1]
                )
                t = ht_pool.tile([P, TOKC], F32, tag="ht", name="ytmp")
                nc.vector.tensor_scalar_mul(
                    t[:], wg_sb[1][:], b2t_sb[1][:, m : m + 1]
                )
                nc.vector.tensor_add(y_sb[:, m, :], y_sb[:, m, :], t[:])

            for i, (e, c) in enumerate(pairs):
                hts, w2s = emit_act(e, c, psums_cur)
                if i + 1 < len(pairs):
                    psums_cur = emit_l1(*pairs[i + 1])
                emit_l2(e, c, hts, w2s)

            yt3 = yt.rearrange("(mo p) t -> p mo t", p=P)
            for m in range(M2):
                nc.sync.dma_start(yt3[:, m, :], y_sb[:, m, :])

    nc.compile()
    return nc


_CACHED = {}


def _get_nc(kind, cap=None):
    key = (kind, cap)
    if key not in _CACHED:
        nc = bacc.Bacc(
            "TRN2",
            target_bir_lowering=False,
            debug=False,
            num_devices=N_CORES,
        )
        if kind == "sparse":
            _CACHED[key] = _build_sparse(nc, cap)
        else:
            _CACHED[key] = _build_dense(nc)
    return _CACHED[key]


def _run(nc, in_maps):
    trace = bool(int(os.environ.get("KERNEL_TRACE", "0")))
    if trace:
        _ensure_ntff_hook()
    res = bass_utils.run_bass_kernel_spmd(
        nc, in_maps, core_ids=list(range(N_CORES)), trace=trace
    )
    if trace:
        kernel.last_exec_time_ns = res.exec_time_ns
        kernel.last_results = res
    return res


def kernel(**inputs):
    x = np.asarray(inputs["x_local"], dtype=np.float32)          # (8192, 2048)
    ids = np.asarray(inputs["top2_exp_id"])                       # (8192, 2)
    tw = np.asarray(inputs["top2_weight"], dtype=np.float32)      # (8192, 2)

    sel = (ids % 2).astype(np.float32)
    wge = [
        (tw * (1.0 - sel)).sum(axis=1).astype(np.float32),        # expert-0 gate
        (tw * sel).sum(axis=1).astype(np.float32),                # expert-1 gate
    ]

    xtb = np.ascontiguousarray(x.T.astype(BF16NP))                # (2048, 8192) bf16

    sparse_shared = {}
    for e in range(2):
        w1b = np.asarray(inputs[f"W1_{e}"], dtype=np.float32).astype(BF16NP)
        # pack into quad-strip blocks [q, c, p, kk, f]: one [P,4,CHUNK]
        # quad per DMA with 4KB contiguous per-partition lines
        sparse_shared[f"w1_{e}"] = np.ascontiguousarray(
            w1b.reshape(KM // 4, 4, P, NCHUNK, CHUNK).transpose(0, 3, 2, 1, 4)
        )
        sparse_shared[f"w2_{e}"] = np.ascontiguousarray(
            np.asarray(inputs[f"W2_{e}"], dtype=np.float32).astype(BF16NP)
        )
        sparse_shared[f"b1t_{e}"] = np.ascontiguousarray(
            np.asarray(inputs[f"b1_{e}"], dtype=np.float32).reshape(D_FF // P, P).T
        )
        sparse_shared[f"b2t_{e}"] = np.ascontiguousarray(
            np.asarray(inputs[f"b2_{e}"], dtype=np.float32).reshape(M2, P).T
        )

    # Globally-balanced gathers: each expert's active set (~75% of all
    # tokens) is split evenly across the 8 cores, so per-core load is
    # |S_e|/8 +- 1 regardless of which core a token "belongs" to.
    # CAP is sized to the observed routing (rounded up to a multiple of
    # 16 so the compiled kernel is reused across calls with similar
    # counts); compile time is per-CAP one-time, off the HW clock.
    glocs = [np.flatnonzero(wge[e] > 0) for e in range(2)]
    maxper = max(-(-len(g) // N_CORES) for g in glocs)
    CAP = max(512, ((maxper + 7) // 8) * 8)
    overflow = CAP > CAP_MAX

    if not overflow:
        splits = [np.array_split(glocs[e], N_CORES) for e in range(2)]
        in_maps = []
        for c in range(N_CORES):
            m = dict(sparse_shared)
            for e in range(2):
                loc = splits[e][c]
                cnt = len(loc)
                xgc = np.zeros((D_MODEL, CAP), BF16NP)
                xgc[:, :cnt] = xtb[:, loc]
                # partition-major [p, k, t]
                m[f"xg{e}"] = np.ascontiguousarray(
                    xgc.reshape(KM, P, CAP).transpose(1, 0, 2)
                )
                wggc = np.zeros((CAP,), np.float32)
                wggc[:cnt] = wge[e][loc]
                m[f"wgg{e}"] = np.ascontiguousarray(
                    np.broadcast_to(wggc, (P, CAP))
                )
            in_maps.append(m)

        res = _run(_get_nc("sparse", CAP), in_maps)

        y = np.zeros((N_LOCAL, D_MODEL), np.float32)
        for c in range(N_CORES):
            for e in range(2):
                loc = splits[e][c]
                cnt = len(loc)
                r = res.results[c][f"yt{e}"]  # (P, M2, CAP) bf16
                yd = r.transpose(1, 0, 2).reshape(D_MODEL, CAP)
                y[loc] += yd.T[:cnt].astype(np.float32)
        return y

    # dense fallback (vanishingly rare: a gather exceeded capacity)
    xt = np.ascontiguousarray(x.T)
    dense_shared = {}
    for e in range(2):
        dense_shared[f"w1_{e}"] = np.ascontiguousarray(
            np.asarray(inputs[f"W1_{e}"], dtype=np.float32)
        )
        dense_shared[f"w2_{e}"] = np.ascontiguousarray(
            np.asarray(inputs[f"W2_{e}"], dtype=np.float32)
        )
        dense_shared[f"b1t_{e}"] = sparse_shared[f"b1t_{e}"]
        dense_shared[f"b2t_{e}"] = sparse_shared[f"b2t_{e}"]
    in_maps = []
    for c in range(N_CORES):
        tok = slice(c * TOKC, (c + 1) * TOKC)
        m = dict(dense_shared)
        m["xt"] = np.ascontiguousarray(xt[:, tok])
        for e in range(2):
            m[f"wg{e}"] = np.ascontiguousarray(
                np.broadcast_to(wge[e][tok], (P, TOKC)).astype(np.float32)
            )
        in_maps.append(m)
    res = _run(_get_nc("dense"), in_maps)
    ytc = np.concatenate([r["yt"] for r in res.results], axis=1)  # (2048, 8192)
    return np.ascontiguousarray(ytc.T)


# revision 57
# speedup vs baseline: 1.7519x; 1.0021x over previous
"""MoE top-2 (2 experts) FFN kernel for TRN2, 8 NeuronCores.

Problem (hardcoded):
  x:   (8192, 2048) f32 tokens
  two expert FFNs: d_model=2048 -> d_ff=8192 (gelu exact) -> 2048
  out[i] = w0[i] * FFN0(x[i]) + w1[i] * FFN1(x[i])
  where w_e[i] = sum of top2_weight[i, s] over slots s with (top2_exp_id[i,s] % 2) == e

Strategy:
  - Host: fold top-2 gating into per-token scalars w0/w1; gather each
    expert's active tokens (those with w_e > 0, ~75% of tokens) with a
    globally balanced split across the 8 cores. Capacity CAP is sized
    per call from the observed routing counts (compile is per-CAP,
    one-time, off the HW clock); dense fallback above CAP_MAX.
  - bf16 weights + activations (fp32 PSUM accumulate, fp32 gelu/y):
    halves HBM traffic and enables FWL so LDWEIGHTS fully hides under
    the 1-cycle/row matmul stream; l2 err ~4e-3 vs the 2e-2 gate.
  - On-core: activations kept transposed ([d_model|d_ff on partitions] x
    [tokens on free dim]) so both matmul layers contract along partitions.
    W1 host-packed into contiguous [P,4,CHUNK] quad-strips (one 512KB DMA
    with 4KB per-partition lines each); xg/yt partition-major.
  - Both experts' gathered xT stay SBUF-resident (~6MB bf16); expert 1's
    tiles prefetch during expert 0 compute -> no stall at the switch.
  - d_ff processed in chunks of 512; layer-2 partials accumulated into an
    SBUF-resident fp32 y; the last chunk's accumulate writes bf16 and
    stores half the bytes. Each weight byte is streamed exactly once.
  - Software-pipelined emission: PE order L1(0),L1(1),L2(0),L1(2),L2(1)...
    so gelu/gate (ACT+DVE) of chunk i overlaps L1(i+1) matmuls. Steady
    state issues matmuls at the N/2.4GHz+2.5ns hardware floor (~96% MFU).
  - Startup: garbage warm-up matmuls flip the HAM clock gate to 8/8 and a
    dummy gelu preloads the ACT table while the first DMAs land; startup
    DMAs are need-ordered across the two HW-DGE queues (sync + scalar),
    keeping the scalar FIFO free ahead of the first gelu (shared DMA ring
    credits would block it and stall the next chunk's PSUM release).
"""

import os

import ml_dtypes
import numpy as np

import concourse.bass as bass
import concourse.mybir as mybir
import concourse.tile as tile
from concourse import bacc
from concourse import bass_utils

BF16NP = ml_dtypes.bfloat16


def _ensure_ntff_hook():
    """This image's `antenv` lacks `axon_hooks`, so boot-time NTFF hook
    install degrades silently and trace=True captures nothing. Register a
    shim module and install the ctypes-driven hook (same as trn_boot)."""
    import sys
    import types

    if "antenv.axon_hooks" in sys.modules:
        return
    mod = types.ModuleType("antenv.axon_hooks")
    mod._hook = None

    def set_axon_ntff_profile_hook(h):
        mod._hook = h

    def get_axon_ntff_profile_hook():
        return mod._hook

    mod.set_axon_ntff_profile_hook = set_axon_ntff_profile_hook
    mod.get_axon_ntff_profile_hook = get_axon_ntff_profile_hook
    sys.modules["antenv.axon_hooks"] = mod
    try:
        from trn_agent_boot.trn_boot import _ntff_profile_via_ctypes

        hook = _ntff_profile_via_ctypes("/opt/axon/libaxon_pjrt.so")
        if hook is not None:
            mod._hook = hook
    except Exception:
        pass


P = 128
D_MODEL = 2048
D_FF = 8192
N_LOCAL = 8192
N_CORES = 8
TOKC = N_LOCAL // N_CORES      # 1024 tokens per core
CAP_MAX = 832                  # per-expert gathered-token capacity limit
                               # (SBUF budget); dense fallback above this.
                               # actual CAP is chosen per call from the
                               # observed routing counts.
KM = D_MODEL // P              # 16 contraction tiles for layer 1
CHUNK = 512                    # d_ff chunk held in PSUM per pass
FC = CHUNK // P                # 4 d_ff tiles per chunk
NCHUNK = D_FF // CHUNK         # 16
M2 = D_MODEL // P              # 16 output d_model tiles

F32 = mybir.dt.float32
F32R = mybir.dt.float32r
BF16 = mybir.dt.bfloat16
GELU = mybir.ActivationFunctionType.Gelu


def _blocks(total):
    """Moving-dim blocks: each <= 512 (PSUM bank limit for f32 output).
    Equal blocks keep per-matmul overhead uniform."""
    n = (total + 511) // 512
    base = total // n
    out = []
    off = 0
    for i in range(n):
        hs = base + (1 if i < total - base * n else 0)
        out.append((off, hs))
        off += hs
    assert off == total and all(hs <= 512 for _, hs in out)
    return out


def _build_sparse(nc, caps):
    """Per-expert gathered tokens (caps[e] per core); expert passes run
    back-to-back; both experts' xT tiles are SBUF-resident."""
    HSs = [_blocks(caps[e]) for e in range(2)]
    CAPM = max(caps)
    # xg host-packed partition-major: [p, k, t] -> per-partition lines of
    # (k1-k0)*cap*2B per group DMA
    xg = [
        nc.dram_tensor(f"xg{e}", (P, KM, caps[e]), BF16,
                       kind="ExternalInput").ap()
        for e in range(2)
    ]
    # w1 host-packed quad-strips: [q, c, p, kk, f] -> a [P, 4, CHUNK] quad
    # is one contiguous 512KB block with 4KB per-partition lines
    w1 = [
        nc.dram_tensor(f"w1_{e}", (KM // 4, NCHUNK, P, 4, CHUNK), BF16,
                       kind="ExternalInput").ap()
        for e in range(2)
    ]
    w2 = [
        nc.dram_tensor(f"w2_{e}", (D_FF, D_MODEL), BF16, kind="ExternalInput").ap()
        for e in range(2)
    ]
    b1t = [
        nc.dram_tensor(f"b1t_{e}", (P, D_FF // P), F32, kind="ExternalInput").ap()
        for e in range(2)
    ]
    b2t = [
        nc.dram_tensor(f"b2t_{e}", (P, M2), F32, kind="ExternalInput").ap()
        for e in range(2)
    ]
    wgg = [
        nc.dram_tensor(f"wgg{e}", (P, caps[e]), F32, kind="ExternalInput").ap()
        for e in range(2)
    ]
    # yt partition-major: [p, m, t]
    yt = [
        nc.dram_tensor(f"yt{e}", (P, M2, caps[e]), BF16,
                       kind="ExternalOutput").ap()
        for e in range(2)
    ]

    with tile.TileContext(nc) as tc:
        with (
            tc.tile_pool(name="const", bufs=1) as const_pool,
            tc.tile_pool(name="w1s", bufs=8) as w1_pool,
            tc.tile_pool(name="w2s", bufs=6) as w2_pool,
            tc.tile_pool(name="ht", bufs=8) as ht_pool,
            tc.tile_pool(name="ps", bufs=8, space="PSUM") as psum_pool,
        ):
            # Both experts' xT k-tiles live in SBUF simultaneously (bf16).
            xt_sb = [
                const_pool.tile([P, KM, caps[e]], BF16, tag=f"xt{e}",
                                name=f"xt_sb{e}")
                for e in range(2)
            ]
            y_sb = const_pool.tile([P, M2, CAPM], F32, tag="y", name="y_sb")
            # final-chunk accumulate lands here in bf16 -> half-size stores
            yb_sb = const_pool.tile([P, M2, CAPM], BF16, tag="yb", name="yb_sb")
            wgg_sb = [
                const_pool.tile([P, caps[e]], F32, tag=f"wgg{e}",
                                name=f"wgg{e}_sb")
                for e in range(2)
            ]
            b1t_sb = [
                const_pool.tile([P, D_FF // P], F32, tag=f"b1t{e}", name=f"b1t{e}_sb")
                for e in range(2)
            ]
            b2t_sb = [
                const_pool.tile([P, M2], F32, tag=f"b2t{e}", name=f"b2t{e}_sb")
                for e in range(2)
            ]

            xg3 = xg
            yt3 = yt

            # chunk schedule: (expert, d_ff tile start fi0, n tiles fc)
            NFI = D_FF // P  # 64
            chunks = [
                (e, c * FC, FC) for e in range(2) for c in range(NCHUNK)
            ]

            # xT k-tile groups: small up front so the first matmuls aren't
            # gated on a big transfer, fat later. Groups MUST be emitted at
            # or before the w1-quad loop that consumes their k range.
            XT_GROUPS = [(0, 1), (1, 2), (2, 4), (4, 8), (8, 12), (12, 16)]

            # warm-up: garbage matmuls while the startup DMAs are in
            # flight, so HAM un-throttles the PE (~3.4us of activity)
            # before the first real matmul
            dummy = const_pool.tile([P, 256], BF16, tag="dummy", name="dummy_sb")
            nc.gpsimd.memset(dummy[:], 0)
            # preload the gelu table (1.3us ACT_TABLE_LOAD) off the
            # critical path, before the first real gelu needs it
            nc.scalar.activation(dummy[:, 128:256], dummy[:, :128], GELU)
            dps = psum_pool.tile([P, 128], F32, tag="ps", name="dummy_ps")
            for _ in range(16):
                nc.tensor.matmul(dps[:], dummy[:, :P], dummy[:, :P], start=True,
                                 stop=True)

            def emit_aux(e):
                # tiny b1t/b2t on scalar (they release ring credits fast
                # and gelu needs b1t); fat wgg goes on sync separately
                nc.scalar.dma_start(b1t_sb[e][:], b1t[e][:])
                nc.scalar.dma_start(b2t_sb[e][:], b2t[e][:])

            def emit_l1(ci, e, fi0, fc, first=False):
                """PE: layer-1 matmuls for one (expert, chunk)."""
                cb, fo = fi0 // FC, (fi0 % FC) * P
                ncol = fc * P
                HS = HSs[e]
                psums = [
                    [
                        psum_pool.tile(
                            [P, hs], F32, tag="ps", name=f"ps1_{e}_{fi0}_{f}_{h}"
                        )
                        for h, (off, hs) in enumerate(HS)
                    ]
                    for f in range(fc)
                ]
                for q in range(KM // 4):
                    # quad-strip w1 tile: 4 k-strips per DMA enqueue
                    w1s = w1_pool.tile(
                        [P, 4, CHUNK], BF16, tag="w1s", name=f"w1s_{e}_{fi0}_{q}"
                    )
                    if first and q == 0:
                        # sub-loads so the first matmuls gate on 64KB
                        for k0, k1 in ((0, 1), (1, 2), (2, 4)):
                            nc.sync.dma_start(
                                w1s[:, k0:k1, :ncol],
                                w1[e][q, cb, :, k0:k1, fo : fo + ncol],
                            )
                    else:
                        nc.sync.dma_start(
                            w1s[:, :, :ncol], w1[e][q, cb, :, :, fo : fo + ncol]
                        )
                    if first:
                        # aux first: tiny, needed by the first gelu; the
                        # ONLY scalar-queue DMAs before the first gelu
                        # (shared ring credits would block it otherwise)
                        if q == 0:
                            emit_aux(0)
                        # expert-0 xT: the two tiny head groups ride the
                        # scalar queue (their rings clear long before the
                        # first gelu), the fat rest interleave with the w1
                        # quads on sync in need-order
                        for gi in ({0: (0, 1, 2), 1: (3,), 2: (4,), 3: (5,)}[q]):
                            k0, k1 = XT_GROUPS[gi]
                            nc.sync.dma_start(
                                xt_sb[0][:, k0:k1, :], xg3[0][:, k0:k1, :]
                            )
                        if q == 3:
                            nc.sync.dma_start(wgg_sb[0][:], wgg[0][:])
                    if ci == 1 and q == 0:
                        emit_aux(1)
                        nc.sync.dma_start(wgg_sb[1][:], wgg[1][:])
                    if 3 <= ci <= 6 and q == 0:
                        # prefetch expert-1 xT (one 4-tile group per chunk)
                        # well before the expert switch, on the scalar queue
                        k0 = 4 * (ci - 3)
                        nc.scalar.dma_start(
                            xt_sb[1][:, k0 : k0 + 4, :], xg3[1][:, k0 : k0 + 4, :]
                        )
                    for kk in range(4):
                        k = 4 * q + kk
                        for f in range(fc):
                            for h, (off, hs) in enumerate(HS):
                                nc.tensor.matmul(
                                    psums[f][h][:],
                                    w1s[:, kk, f * P : (f + 1) * P],
                                    xt_sb[e][:, k, off : off + hs],
                                    start=(k == 0),
                                    stop=(k == KM - 1),
                                )
                return psums

            def emit_act(e, fi0, fc, psums, first=False):
                """ACT+DVE: gelu(+b1), gate scale. Also W2 strip loads
                (scalar queue, enqueued BEFORE the gelu ops so they don't
                wait behind them in the FIFO — except the very first chunk,
                where the startup DMA crunch would exhaust ring credits and
                block the first gelu, stalling the next chunk's PSUM
                release), and (on each expert's first chunk) the gated b2
                y-init."""
                def emit_w2():
                    w2s = []
                    for f in range(fc):
                        w2f = w2_pool.tile(
                            [P, D_MODEL], BF16, tag="w2s",
                            name=f"w2s_{e}_{fi0}_{f}"
                        )
                        row = (fi0 + f) * P
                        nc.scalar.dma_start(w2f[:], w2[e][row : row + P, :])
                        w2s.append(w2f)
                    return w2s

                HS = HSs[e]
                w2s = None if first else emit_w2()
                if fi0 == 0:
                    for m in range(M2):
                        nc.vector.tensor_scalar_mul(
                            y_sb[:, m, : caps[e]], wgg_sb[e][:],
                            b2t_sb[e][:, m : m + 1]
                        )
                hts = []
                for f in range(fc):
                    ht = ht_pool.tile(
                        [P, caps[e]], BF16, tag="ht", name=f"ht_{e}_{fi0}_{f}"
                    )
                    col = fi0 + f
                    for h, (off, hs) in enumerate(HS):
                        nc.scalar.activation(
                            ht[:, off : off + hs],
                            psums[f][h][:],
                            GELU,
                            bias=b1t_sb[e][:, col : col + 1],
                        )
                    nc.vector.tensor_mul(ht[:], ht[:], wgg_sb[e][:])
                    hts.append(ht)
                if w2s is None:
                    w2s = emit_w2()
                return hts, w2s

            def emit_l2(e, fi0, fc, hts, w2s):
                """PE: layer-2 matmuls; DVE: accumulate into y; the last
                chunk's accumulate writes bf16 and stores half the bytes."""
                HS = HSs[e]
                last = fi0 + fc == NFI
                for m in range(M2):
                    for h, (off, hs) in enumerate(HS):
                        ps = psum_pool.tile(
                            [P, hs], F32, tag="ps", name=f"ps2_{e}_{fi0}_{m}_{h}"
                        )
                        for f in range(fc):
                            nc.tensor.matmul(
                                ps[:],
                                w2s[f][:, m * P : (m + 1) * P],
                                hts[f][:, off : off + hs],
                                start=(f == 0),
                                stop=(f == fc - 1),
                            )
                        ysl = y_sb[:, m, off : off + hs]
                        if last:
                            nc.vector.tensor_add(
                                yb_sb[:, m, off : off + hs], ysl, ps[:]
                            )
                        else:
                            nc.vector.tensor_add(ysl, ysl, ps[:])
                    if last:
                        # split the final stores across both HW-DGE queues
                        eng = nc.sync if m % 2 == 0 else nc.scalar
                        eng.dma_start(
                            yt3[e][:, m, :], yb_sb[:, m, : caps[e]]
                        )

            psums_cur = emit_l1(0, *chunks[0], first=True)
            for i, (e, fi0, fc) in enumerate(chunks):
                hts, w2s = emit_act(e, fi0, fc, psums_cur, first=(i == 0))
                if i + 1 < len(chunks):
                    psums_cur = emit_l1(i + 1, *chunks[i + 1])
                emit_l2(e, fi0, fc, hts, w2s)

    nc.compile()
    return nc


def _build_dense(nc):
    """Dense fallback: both experts over all tokens, gate-weighted."""
    HS = [(0, 512), (512, 512)]
    xt = nc.dram_tensor("xt", (D_MODEL, TOKC), F32R, kind="ExternalInput").ap()
    w1 = [
        nc.dram_tensor(f"w1_{e}", (D_MODEL, D_FF), F32R, kind="ExternalInput").ap()
        for e in range(2)
    ]
    w2 = [
        nc.dram_tensor(f"w2_{e}", (D_FF, D_MODEL), F32R, kind="ExternalInput").ap()
        for e in range(2)
    ]
    b1t = [
        nc.dram_tensor(f"b1t_{e}", (P, D_FF // P), F32, kind="ExternalInput").ap()
        for e in range(2)
    ]
    b2t = [
        nc.dram_tensor(f"b2t_{e}", (P, M2), F32, kind="ExternalInput").ap()
        for e in range(2)
    ]
    wg = [
        nc.dram_tensor(f"wg{e}", (P, TOKC), F32, kind="ExternalInput").ap()
        for e in range(2)
    ]
    yt = nc.dram_tensor("yt", (D_MODEL, TOKC), F32, kind="ExternalOutput").ap()

    with tile.TileContext(nc) as tc:
        with (
            tc.tile_pool(name="const", bufs=1) as const_pool,
            tc.tile_pool(name="w1s", bufs=5) as w1_pool,
            tc.tile_pool(name="w2s", bufs=5) as w2_pool,
            tc.tile_pool(name="ht", bufs=5) as ht_pool,
            tc.tile_pool(name="ps", bufs=8, space="PSUM") as psum_pool,
        ):
            xt_sb = const_pool.tile([P, KM, TOKC], F32R, tag="xt", name="xt_sb")
            y_sb = const_pool.tile([P, M2, TOKC], F32, tag="y", name="y_sb")
            wg_sb = [
                const_pool.tile([P, TOKC], F32, tag=f"wg{e}", name=f"wg{e}_sb")
                for e in range(2)
            ]
            b1t_sb = [
                const_pool.tile([P, D_FF // P], F32, tag=f"b1t{e}", name=f"b1t{e}_sb")
                for e in range(2)
            ]
            b2t_sb = [
                const_pool.tile([P, M2], F32, tag=f"b2t{e}", name=f"b2t{e}_sb")
                for e in range(2)
            ]

            xt3 = xt.rearrange("(ko p) t -> p ko t", p=P)
            pairs = [(e, c) for e in range(2) for c in range(NCHUNK)]

            def emit_l1(e, c, first=False):
                psums = [
                    [
                        psum_pool.tile(
                            [P, hs], F32, tag="ps", name=f"ps1_{e}_{c}_{f}_{h}"
                        )
                        for h, (off, hs) in enumerate(HS)
                    ]
                    for f in range(FC)
                ]
                for k in range(KM):
                    if first:
                        nc.sync.dma_start(xt_sb[:, k, :], xt3[:, k, :])
                        if k == 0:
                            for ee in range(2):
                                nc.sync.dma_start(wg_sb[ee][:], wg[ee][:])
                                nc.sync.dma_start(b1t_sb[ee][:], b1t[ee][:])
                                nc.sync.dma_start(b2t_sb[ee][:], b2t[ee][:])
                    w1s = w1_pool.tile(
                        [P, CHUNK], F32R, tag="w1s", name=f"w1s_{e}_{c}_{k}"
                    )
                    nc.sync.dma_start(
                        w1s[:],
                        w1[e][k * P : (k + 1) * P, c * CHUNK : (c + 1) * CHUNK],
                    )
                    for f in range(FC):
                        for h, (off, hs) in enumerate(HS):
                            nc.tensor.matmul(
                                psums[f][h][:],
                                w1s[:, f * P : (f + 1) * P],
                                xt_sb[:, k, off : off + hs],
                                start=(k == 0),
                                stop=(k == KM - 1),
                            )
                return psums

            def emit_act(e, c, psums):
                hts = []
                for f in range(FC):
                    ht = ht_pool.tile(
                        [P, TOKC], F32R, tag="ht", name=f"ht_{e}_{c}_{f}"
                    )
                    col = c * FC + f
                    for h, (off, hs) in enumerate(HS):
                        nc.scalar.activation(
                            ht[:, off : off + hs],
                            psums[f][h][:],
                            GELU,
                            bias=b1t_sb[e][:, col : col + 1],
                        )
                    nc.vector.tensor_mul(ht[:], ht[:], wg_sb[e][:])
                    hts.append(ht)
                w2s = []
                for f in range(FC):
                    w2f = w2_pool.tile(
                        [P, D_MODEL], F32R, tag="w2s", name=f"w2s_{e}_{c}_{f}"
                    )
                    row = (c * FC + f) * P
                    nc.sync.dma_start(w2f[:], w2[e][row : row + P, :])
                    w2s.append(w2f)
                return hts, w2s

            def emit_l2(e, c, hts, w2s):
                for m in range(M2):
                    for h, (off, hs) in enumerate(HS):
                        ps = psum_pool.tile(
                            [P, hs], F32, tag="ps", name=f"ps2_{e}_{c}_{m}_{h}"
                        )
                        for f in range(FC):
                            nc.tensor.matmul(
                                ps[:],
                                w2s[f][:, m * P : (m + 1) * P],
                                hts[f][:, off : off + hs],
                                start=(f == 0),
                                stop=(f == FC - 1),
                            )
                        ysl = y_sb[:, m, off : off + hs]
                        nc.vector.tensor_add(ysl, ysl, ps[:])

            psums_cur = emit_l1(*pairs[0], first=True)

            for m in range(M2):
                nc.vector.tensor_scalar_mul(
                    y_sb[:, m, :], wg_sb[0][:], b2t_sb[0][:, m : m + 1]
                )
                t = ht_pool.tile([P, TOKC], F32, tag="ht", name="ytmp")
                nc.vector.tensor_scalar_mul(
                    t[:], wg_sb[1][:], b2t_sb[1][:, m : m + 1]
                )
                nc.vector.tensor_add(y_sb[:, m, :], y_sb[:, m, :], t[:])

            for i, (e, c) in enumerate(pairs):
                hts, w2s = emit_act(e, c, psums_cur)
                if i + 1 < len(pairs):
                    psums_cur = emit_l1(*pairs[i + 1])
                emit_l2(e, c, hts, w2s)

            yt3 = yt.rearrange("(mo p) t -> p mo t", p=P)
            for m in range(M2):
                nc.sync.dma_start(yt3[:, m, :], y_sb[:, m, :])

    nc.compile()
    return nc


_CACHED = {}


def _get_nc(kind, cap=None):
    key = (kind, cap)
    if key not in _CACHED:
        nc = bacc.Bacc(
            "TRN2",
            target_bir_lowering=False,
            debug=False,
            num_devices=N_CORES,
        )
        if kind == "sparse":
            _CACHED[key] = _build_sparse(nc, cap)
        else:
            _CACHED[key] = _build_dense(nc)
    return _CACHED[key]


def _run(nc, in_maps):
    trace = bool(int(os.environ.get("KERNEL_TRACE", "0")))
    if trace:
        _ensure_ntff_hook()
    res = bass_utils.run_bass_kernel_spmd(
        nc, in_maps, core_ids=list(range(N_CORES)), trace=trace
    )
    if trace:
        kernel.last_exec_time_ns = res.exec_time_ns
        kernel.last_results = res
    return res


def kernel(**inputs):
    x = np.asarray(inputs["x_local"], dtype=np.float32)          # (8192, 2048)
    ids = np.asarray(inputs["top2_exp_id"])                       # (8192, 2)
    tw = np.asarray(inputs["top2_weight"], dtype=np.float32)      # (8192, 2)

    sel = (ids % 2).astype(np.float32)
    wge = [
        (tw * (1.0 - sel)).sum(axis=1).astype(np.float32),        # expert-0 gate
        (tw * sel).sum(axis=1).astype(np.float32),                # expert-1 gate
    ]

    xtb = np.ascontiguousarray(x.T.astype(BF16NP))                # (2048, 8192) bf16

    sparse_shared = {}
    for e in range(2):
        w1b = np.asarray(inputs[f"W1_{e}"], dtype=np.float32).astype(BF16NP)
        # pack into quad-strip blocks [q, c, p, kk, f]: one [P,4,CHUNK]
        # quad per DMA with 4KB contiguous per-partition lines
        sparse_shared[f"w1_{e}"] = np.ascontiguousarray(
            w1b.reshape(KM // 4, 4, P, NCHUNK, CHUNK).transpose(0, 3, 2, 1, 4)
        )
        sparse_shared[f"w2_{e}"] = np.ascontiguousarray(
            np.asarray(inputs[f"W2_{e}"], dtype=np.float32).astype(BF16NP)
        )
        sparse_shared[f"b1t_{e}"] = np.ascontiguousarray(
            np.asarray(inputs[f"b1_{e}"], dtype=np.float32).reshape(D_FF // P, P).T
        )
        sparse_shared[f"b2t_{e}"] = np.ascontiguousarray(
            np.asarray(inputs[f"b2_{e}"], dtype=np.float32).reshape(M2, P).T
        )

    # Globally-balanced gathers: each expert's active set (~75% of all
    # tokens) is split evenly across the 8 cores, so per-core load is
    # |S_e|/8 +- 1 regardless of which core a token "belongs" to.
    # CAP is sized to the observed routing (rounded up to a multiple of
    # 16 so the compiled kernel is reused across calls with similar
    # counts); compile time is per-CAP one-time, off the HW clock.
    glocs = [np.flatnonzero(wge[e] > 0) for e in range(2)]
    caps = tuple(
        max(512, ((-(-len(g) // N_CORES) + 7) // 8) * 8) for g in glocs
    )
    overflow = max(caps) > CAP_MAX

    if not overflow:
        splits = [np.array_split(glocs[e], N_CORES) for e in range(2)]
        in_maps = []
        for c in range(N_CORES):
            m = dict(sparse_shared)
            for e in range(2):
                cap = caps[e]
                loc = splits[e][c]
                cnt = len(loc)
                xgc = np.zeros((D_MODEL, cap), BF16NP)
                xgc[:, :cnt] = xtb[:, loc]
                # partition-major [p, k, t]
                m[f"xg{e}"] = np.ascontiguousarray(
                    xgc.reshape(KM, P, cap).transpose(1, 0, 2)
                )
                wggc = np.zeros((cap,), np.float32)
                wggc[:cnt] = wge[e][loc]
                m[f"wgg{e}"] = np.ascontiguousarray(
                    np.broadcast_to(wggc, (P, cap))
                )
            in_maps.append(m)

        res = _run(_get_nc("sparse", caps), in_maps)

        y = np.zeros((N_LOCAL, D_MODEL), np.float32)
        for c in range(N_CORES):
            for e in range(2):
                loc = splits[e][c]
                cnt = len(loc)
                r = res.results[c][f"yt{e}"]  # (P, M2, cap) bf16
                yd = r.transpose(1, 0, 2).reshape(D_MODEL, caps[e])
                y[loc] += yd.T[:cnt].astype(np.float32)
        return y

    # dense fallback (vanishingly rare: a gather exceeded capacity)
    xt = np.ascontiguousarray(x.T)
    dense_shared = {}
    for e in range(2):
        dense_shared[f"w1_{e}"] = np.ascontiguousarray(
            np.asarray(inputs[f"W1_{e}"], dtype=np.float32)
        )
        dense_shared[f"w2_{e}"] = np.ascontiguousarray(
            np.asarray(inputs[f"W2_{e}"], dtype=np.float32)
        )
        dense_shared[f"b1t_{e}"] = sparse_shared[f"b1t_{e}"]
        dense_shared[f"b2t_{e}"] = sparse_shared[f"b2t_{e}"]
    in_maps = []
    for c in range(N_CORES):
        tok = slice(c * TOKC, (c + 1) * TOKC)
        m = dict(dense_shared)
        m["xt"] = np.ascontiguousarray(xt[:, tok])
        for e in range(2):
            m[f"wg{e}"] = np.ascontiguousarray(
                np.broadcast_to(wge[e][tok], (P, TOKC)).astype(np.float32)
            )
        in_maps.append(m)
    res = _run(_get_nc("dense"), in_maps)
    ytc = np.concatenate([r["yt"] for r in res.results], axis=1)  # (2048, 8192)
    return np.ascontiguousarray(ytc.T)


# revision 58
# speedup vs baseline: 1.7545x; 1.0015x over previous
"""MoE top-2 (2 experts) FFN kernel for TRN2, 8 NeuronCores.

Problem (hardcoded):
  x:   (8192, 2048) f32 tokens
  two expert FFNs: d_model=2048 -> d_ff=8192 (gelu exact) -> 2048
  out[i] = w0[i] * FFN0(x[i]) + w1[i] * FFN1(x[i])
  where w_e[i] = sum of top2_weight[i, s] over slots s with (top2_exp_id[i,s] % 2) == e

Strategy:
  - Host: fold top-2 gating into per-token scalars w0/w1; gather each
    expert's active tokens (those with w_e > 0, ~75% of tokens) with a
    globally balanced split across the 8 cores. Capacity CAP is sized
    per call from the observed routing counts (compile is per-CAP,
    one-time, off the HW clock); dense fallback above CAP_MAX.
  - bf16 weights + activations (fp32 PSUM accumulate, fp32 gelu/y):
    halves HBM traffic and enables FWL so LDWEIGHTS fully hides under
    the 1-cycle/row matmul stream; l2 err ~4e-3 vs the 2e-2 gate.
  - On-core: activations kept transposed ([d_model|d_ff on partitions] x
    [tokens on free dim]) so both matmul layers contract along partitions.
    W1 host-packed into contiguous [P,4,CHUNK] quad-strips (one 512KB DMA
    with 4KB per-partition lines each); xg/yt partition-major.
  - Both experts' gathered xT stay SBUF-resident (~6MB bf16); expert 1's
    tiles prefetch during expert 0 compute -> no stall at the switch.
  - d_ff processed in chunks of 512; layer-2 partials accumulated into an
    SBUF-resident fp32 y; the last chunk's accumulate writes bf16 and
    stores half the bytes. Each weight byte is streamed exactly once.
  - Software-pipelined emission: PE order L1(0),L1(1),L2(0),L1(2),L2(1)...
    so gelu/gate (ACT+DVE) of chunk i overlaps L1(i+1) matmuls. Steady
    state issues matmuls at the N/2.4GHz+2.5ns hardware floor (~96% MFU).
  - Startup: garbage warm-up matmuls flip the HAM clock gate to 8/8 and a
    dummy gelu preloads the ACT table while the first DMAs land; startup
    DMAs are need-ordered across the two HW-DGE queues (sync + scalar),
    keeping the scalar FIFO free ahead of the first gelu (shared DMA ring
    credits would block it and stall the next chunk's PSUM release).
"""

import os

import ml_dtypes
import numpy as np

import concourse.bass as bass
import concourse.mybir as mybir
import concourse.tile as tile
from concourse import bacc
from concourse import bass_utils

BF16NP = ml_dtypes.bfloat16


def _ensure_ntff_hook():
    """This image's `antenv` lacks `axon_hooks`, so boot-time NTFF hook
    install degrades silently and trace=True captures nothing. Register a
    shim module and install the ctypes-driven hook (same as trn_boot)."""
    import sys
    import types

    if "antenv.axon_hooks" in sys.modules:
        return
    mod = types.ModuleType("antenv.axon_hooks")
    mod._hook = None

    def set_axon_ntff_profile_hook(h):
        mod._hook = h

    def get_axon_ntff_profile_hook():
        return mod._hook

    mod.set_axon_ntff_profile_hook = set_axon_ntff_profile_hook
    mod.get_axon_ntff_profile_hook = get_axon_ntff_profile_hook
    sys.modules["antenv.axon_hooks"] = mod
    try:
        from trn_agent_boot.trn_boot import _ntff_profile_via_ctypes

        hook = _ntff_profile_via_ctypes("/opt/axon/libaxon_pjrt.so")
        if hook is not None:
            mod._hook = hook
    except Exception:
        pass


P = 128
D_MODEL = 2048
D_FF = 8192
N_LOCAL = 8192
N_CORES = 8
TOKC = N_LOCAL // N_CORES      # 1024 tokens per core
CAP_MAX = 832                  # per-expert gathered-token capacity limit
                               # (SBUF budget); dense fallback above this.
                               # actual CAP is chosen per call from the
                               # observed routing counts.
KM = D_MODEL // P              # 16 contraction tiles for layer 1
CHUNK = 512                    # d_ff chunk held in PSUM per pass
FC = CHUNK // P                # 4 d_ff tiles per chunk
NCHUNK = D_FF // CHUNK         # 16
M2 = D_MODEL // P              # 16 output d_model tiles

F32 = mybir.dt.float32
F32R = mybir.dt.float32r
BF16 = mybir.dt.bfloat16
GELU = mybir.ActivationFunctionType.Gelu


def _blocks(total):
    """Moving-dim blocks: each <= 512 (PSUM bank limit for f32 output).
    Equal blocks keep per-matmul overhead uniform."""
    n = (total + 511) // 512
    base = total // n
    out = []
    off = 0
    for i in range(n):
        hs = base + (1 if i < total - base * n else 0)
        out.append((off, hs))
        off += hs
    assert off == total and all(hs <= 512 for _, hs in out)
    return out


def _build_sparse(nc, caps):
    """Per-expert gathered tokens (caps[e] per core); expert passes run
    back-to-back; both experts' xT tiles are SBUF-resident."""
    HSs = [_blocks(caps[e]) for e in range(2)]
    CAPM = max(caps)
    # xg host-packed partition-major: [p, k, t] -> per-partition lines of
    # (k1-k0)*cap*2B per group DMA
    xg = [
        nc.dram_tensor(f"xg{e}", (P, KM, caps[e]), BF16,
                       kind="ExternalInput").ap()
        for e in range(2)
    ]
    # w1 host-packed quad-strips: [q, c, p, kk, f] -> a [P, 4, CHUNK] quad
    # is one contiguous 512KB block with 4KB per-partition lines
    w1 = [
        nc.dram_tensor(f"w1_{e}", (KM // 4, NCHUNK, P, 4, CHUNK), BF16,
                       kind="ExternalInput").ap()
        for e in range(2)
    ]
    w2 = [
        nc.dram_tensor(f"w2_{e}", (D_FF, D_MODEL), BF16, kind="ExternalInput").ap()
        for e in range(2)
    ]
    b1t = [
        nc.dram_tensor(f"b1t_{e}", (P, D_FF // P), F32, kind="ExternalInput").ap()
        for e in range(2)
    ]
    b2t = [
        nc.dram_tensor(f"b2t_{e}", (P, M2), F32, kind="ExternalInput").ap()
        for e in range(2)
    ]
    wgg = [
        nc.dram_tensor(f"wgg{e}", (P, caps[e]), F32, kind="ExternalInput").ap()
        for e in range(2)
    ]
    # yt partition-major: [p, m, t]
    yt = [
        nc.dram_tensor(f"yt{e}", (P, M2, caps[e]), BF16,
                       kind="ExternalOutput").ap()
        for e in range(2)
    ]

    with tile.TileContext(nc) as tc:
        with (
            tc.tile_pool(name="const", bufs=1) as const_pool,
            tc.tile_pool(name="w1s", bufs=8) as w1_pool,
            tc.tile_pool(name="w2s", bufs=6) as w2_pool,
            tc.tile_pool(name="ht", bufs=8) as ht_pool,
            tc.tile_pool(name="ps", bufs=8, space="PSUM") as psum_pool,
        ):
            # Both experts' xT k-tiles live in SBUF simultaneously (bf16).
            xt_sb = [
                const_pool.tile([P, KM, caps[e]], BF16, tag=f"xt{e}",
                                name=f"xt_sb{e}")
                for e in range(2)
            ]
            y_sb = const_pool.tile([P, M2, CAPM], F32, tag="y", name="y_sb")
            # final-chunk accumulate lands here in bf16 -> half-size stores
            yb_sb = const_pool.tile([P, M2, CAPM], BF16, tag="yb", name="yb_sb")
            wgg_sb = [
                const_pool.tile([P, caps[e]], F32, tag=f"wgg{e}",
                                name=f"wgg{e}_sb")
                for e in range(2)
            ]
            b1t_sb = [
                const_pool.tile([P, D_FF // P], F32, tag=f"b1t{e}", name=f"b1t{e}_sb")
                for e in range(2)
            ]
            b2t_sb = [
                const_pool.tile([P, M2], F32, tag=f"b2t{e}", name=f"b2t{e}_sb")
                for e in range(2)
            ]

            xg3 = xg
            yt3 = yt

            # chunk schedule: (expert, d_ff tile start fi0, n tiles fc)
            NFI = D_FF // P  # 64
            chunks = [
                (e, c * FC, FC) for e in range(2) for c in range(NCHUNK)
            ]

            # xT k-tile groups: small up front so the first matmuls aren't
            # gated on a big transfer, fat later. Groups MUST be emitted at
            # or before the w1-quad loop that consumes their k range.
            XT_GROUPS = [(0, 1), (1, 2), (2, 4), (4, 8), (8, 12), (12, 16)]

            # warm-up: garbage matmuls while the startup DMAs are in
            # flight, so HAM un-throttles the PE (~3.4us of activity)
            # before the first real matmul
            dummy = const_pool.tile([P, 256], BF16, tag="dummy", name="dummy_sb")
            nc.gpsimd.memset(dummy[:], 0)
            # preload the gelu table (1.3us ACT_TABLE_LOAD) off the
            # critical path, before the first real gelu needs it
            nc.scalar.activation(dummy[:, 128:256], dummy[:, :128], GELU)
            dps = psum_pool.tile([P, 128], F32, tag="ps", name="dummy_ps")

            def warm(n):
                for _ in range(n):
                    nc.tensor.matmul(dps[:], dummy[:, :P], dummy[:, :P],
                                     start=True, stop=True)

            warm(28)

            def emit_aux(e):
                # tiny b1t/b2t on scalar (they release ring credits fast
                # and gelu needs b1t); fat wgg goes on sync separately
                nc.scalar.dma_start(b1t_sb[e][:], b1t[e][:])
                nc.scalar.dma_start(b2t_sb[e][:], b2t[e][:])

            def emit_l1(ci, e, fi0, fc, first=False):
                """PE: layer-1 matmuls for one (expert, chunk)."""
                cb, fo = fi0 // FC, (fi0 % FC) * P
                ncol = fc * P
                HS = HSs[e]
                psums = [
                    [
                        psum_pool.tile(
                            [P, hs], F32, tag="ps", name=f"ps1_{e}_{fi0}_{f}_{h}"
                        )
                        for h, (off, hs) in enumerate(HS)
                    ]
                    for f in range(fc)
                ]
                for q in range(KM // 4):
                    # quad-strip w1 tile: 4 k-strips per DMA enqueue
                    w1s = w1_pool.tile(
                        [P, 4, CHUNK], BF16, tag="w1s", name=f"w1s_{e}_{fi0}_{q}"
                    )
                    if first and q == 0:
                        # sub-loads so the first matmuls gate on 64KB
                        for k0, k1 in ((0, 1), (1, 2), (2, 4)):
                            nc.sync.dma_start(
                                w1s[:, k0:k1, :ncol],
                                w1[e][q, cb, :, k0:k1, fo : fo + ncol],
                            )
                    else:
                        nc.sync.dma_start(
                            w1s[:, :, :ncol], w1[e][q, cb, :, :, fo : fo + ncol]
                        )
                    if first:
                        # aux first: tiny, needed by the first gelu; the
                        # ONLY scalar-queue DMAs before the first gelu
                        # (shared ring credits would block it otherwise)
                        if q == 0:
                            emit_aux(0)
                        # expert-0 xT: the two tiny head groups ride the
                        # scalar queue (their rings clear long before the
                        # first gelu), the fat rest interleave with the w1
                        # quads on sync in need-order
                        for gi in ({0: (0, 1, 2), 1: (3,), 2: (4,), 3: (5,)}[q]):
                            k0, k1 = XT_GROUPS[gi]
                            nc.sync.dma_start(
                                xt_sb[0][:, k0:k1, :], xg3[0][:, k0:k1, :]
                            )
                        if q == 3:
                            nc.sync.dma_start(wgg_sb[0][:], wgg[0][:])
                    if ci == 1 and q == 0:
                        emit_aux(1)
                        nc.sync.dma_start(wgg_sb[1][:], wgg[1][:])
                    if 3 <= ci <= 6 and q == 0:
                        # prefetch expert-1 xT (one 4-tile group per chunk)
                        # well before the expert switch, on the scalar queue
                        k0 = 4 * (ci - 3)
                        nc.scalar.dma_start(
                            xt_sb[1][:, k0 : k0 + 4, :], xg3[1][:, k0 : k0 + 4, :]
                        )
                    for kk in range(4):
                        k = 4 * q + kk
                        for f in range(fc):
                            for h, (off, hs) in enumerate(HS):
                                nc.tensor.matmul(
                                    psums[f][h][:],
                                    w1s[:, kk, f * P : (f + 1) * P],
                                    xt_sb[e][:, k, off : off + hs],
                                    start=(k == 0),
                                    stop=(k == KM - 1),
                                )
                    if ci == 0:
                        # keep HAM warm through the startup DMA trickle:
                        # these run in PE-idle gaps and cost ~0.4us each
                        # batch when data is on time
                        warm(8 if q == 3 else 6)
                return psums

            def emit_act(e, fi0, fc, psums, first=False):
                """ACT+DVE: gelu(+b1), gate scale. Also W2 strip loads
                (scalar queue, enqueued BEFORE the gelu ops so they don't
                wait behind them in the FIFO — except the very first chunk,
                where the startup DMA crunch would exhaust ring credits and
                block the first gelu, stalling the next chunk's PSUM
                release), and (on each expert's first chunk) the gated b2
                y-init."""
                def emit_w2():
                    w2s = []
                    for f in range(fc):
                        w2f = w2_pool.tile(
                            [P, D_MODEL], BF16, tag="w2s",
                            name=f"w2s_{e}_{fi0}_{f}"
                        )
                        row = (fi0 + f) * P
                        nc.scalar.dma_start(w2f[:], w2[e][row : row + P, :])
                        w2s.append(w2f)
                    return w2s

                HS = HSs[e]
                w2s = None if first else emit_w2()
                if fi0 == 0:
                    for m in range(M2):
                        nc.vector.tensor_scalar_mul(
                            y_sb[:, m, : caps[e]], wgg_sb[e][:],
                            b2t_sb[e][:, m : m + 1]
                        )
                hts = []
                for f in range(fc):
                    ht = ht_pool.tile(
                        [P, caps[e]], BF16, tag="ht", name=f"ht_{e}_{fi0}_{f}"
                    )
                    col = fi0 + f
                    for h, (off, hs) in enumerate(HS):
                        nc.scalar.activation(
                            ht[:, off : off + hs],
                            psums[f][h][:],
                            GELU,
                            bias=b1t_sb[e][:, col : col + 1],
                        )
                    nc.vector.tensor_mul(ht[:], ht[:], wgg_sb[e][:])
                    hts.append(ht)
                if w2s is None:
                    w2s = emit_w2()
                return hts, w2s

            def emit_l2(e, fi0, fc, hts, w2s):
                """PE: layer-2 matmuls; DVE: accumulate into y; the last
                chunk's accumulate writes bf16 and stores half the bytes."""
                HS = HSs[e]
                last = fi0 + fc == NFI
                for m in range(M2):
                    for h, (off, hs) in enumerate(HS):
                        ps = psum_pool.tile(
                            [P, hs], F32, tag="ps", name=f"ps2_{e}_{fi0}_{m}_{h}"
                        )
                        for f in range(fc):
                            nc.tensor.matmul(
                                ps[:],
                                w2s[f][:, m * P : (m + 1) * P],
                                hts[f][:, off : off + hs],
                                start=(f == 0),
                                stop=(f == fc - 1),
                            )
                        ysl = y_sb[:, m, off : off + hs]
                        if last:
                            nc.vector.tensor_add(
                                yb_sb[:, m, off : off + hs], ysl, ps[:]
                            )
                        else:
                            nc.vector.tensor_add(ysl, ysl, ps[:])
                    if last:
                        # split the final stores across both HW-DGE queues
                        eng = nc.sync if m % 2 == 0 else nc.scalar
                        eng.dma_start(
                            yt3[e][:, m, :], yb_sb[:, m, : caps[e]]
                        )

            psums_cur = emit_l1(0, *chunks[0], first=True)
            for i, (e, fi0, fc) in enumerate(chunks):
                hts, w2s = emit_act(e, fi0, fc, psums_cur, first=(i == 0))
                if i + 1 < len(chunks):
                    psums_cur = emit_l1(i + 1, *chunks[i + 1])
                emit_l2(e, fi0, fc, hts, w2s)

    nc.compile()
    return nc


def _build_dense(nc):
    """Dense fallback: both experts over all tokens, gate-weighted."""
    HS = [(0, 512), (512, 512)]
    xt = nc.dram_tensor("xt", (D_MODEL, TOKC), F32R, kind="ExternalInput").ap()
    w1 = [
        nc.dram_tensor(f"w1_{e}", (D_MODEL, D_FF), F32R, kind="ExternalInput").ap()
        for e in range(2)
    ]
    w2 = [
        nc.dram_tensor(f"w2_{e}", (D_FF, D_MODEL), F32R, kind="ExternalInput").ap()
        for e in range(2)
    ]
    b1t = [
        nc.dram_tensor(f"b1t_{e}", (P, D_FF // P), F32, kind="ExternalInput").ap()
        for e in range(2)
    ]
    b2t = [
        nc.dram_tensor(f"b2t_{e}", (P, M2), F32, kind="ExternalInput").ap()
        for e in range(2)
    ]
    wg = [
        nc.dram_tensor(f"wg{e}", (P, TOKC), F32, kind="ExternalInput").ap()
        for e in range(2)
    ]
    yt = nc.dram_tensor("yt", (D_MODEL, TOKC), F32, kind="ExternalOutput").ap()

    with tile.TileContext(nc) as tc:
        with (
            tc.tile_pool(name="const", bufs=1) as const_pool,
            tc.tile_pool(name="w1s", bufs=5) as w1_pool,
            tc.tile_pool(name="w2s", bufs=5) as w2_pool,
            tc.tile_pool(name="ht", bufs=5) as ht_pool,
            tc.tile_pool(name="ps", bufs=8, space="PSUM") as psum_pool,
        ):
            xt_sb = const_pool.tile([P, KM, TOKC], F32R, tag="xt", name="xt_sb")
            y_sb = const_pool.tile([P, M2, TOKC], F32, tag="y", name="y_sb")
            wg_sb = [
                const_pool.tile([P, TOKC], F32, tag=f"wg{e}", name=f"wg{e}_sb")
                for e in range(2)
            ]
            b1t_sb = [
                const_pool.tile([P, D_FF // P], F32, tag=f"b1t{e}", name=f"b1t{e}_sb")
                for e in range(2)
            ]
            b2t_sb = [
                const_pool.tile([P, M2], F32, tag=f"b2t{e}", name=f"b2t{e}_sb")
                for e in range(2)
            ]

            xt3 = xt.rearrange("(ko p) t -> p ko t", p=P)
            pairs = [(e, c) for e in range(2) for c in range(NCHUNK)]

            def emit_l1(e, c, first=False):
                psums = [
                    [
                        psum_pool.tile(
                            [P, hs], F32, tag="ps", name=f"ps1_{e}_{c}_{f}_{h}"
                        )
                        for h, (off, hs) in enumerate(HS)
                    ]
                    for f in range(FC)
                ]
                for k in range(KM):
                    if first:
                        nc.sync.dma_start(xt_sb[:, k, :], xt3[:, k, :])
                        if k == 0:
                            for ee in range(2):
                                nc.sync.dma_start(wg_sb[ee][:], wg[ee][:])
                                nc.sync.dma_start(b1t_sb[ee][:], b1t[ee][:])
                                nc.sync.dma_start(b2t_sb[ee][:], b2t[ee][:])
                    w1s = w1_pool.tile(
                        [P, CHUNK], F32R, tag="w1s", name=f"w1s_{e}_{c}_{k}"
                    )
                    nc.sync.dma_start(
                        w1s[:],
                        w1[e][k * P : (k + 1) * P, c * CHUNK : (c + 1) * CHUNK],
                    )
                    for f in range(FC):
                        for h, (off, hs) in enumerate(HS):
                            nc.tensor.matmul(
                                psums[f][h][:],
                                w1s[:, f * P : (f + 1) * P],
                                xt_sb[:, k, off : off + hs],
                                start=(k == 0),
                                stop=(k == KM - 1),
                            )
                return psums

            def emit_act(e, c, psums):
                hts = []
                for f in range(FC):
                    ht = ht_pool.tile(
                        [P, TOKC], F32R, tag="ht", name=f"ht_{e}_{c}_{f}"
                    )
                    col = c * FC + f
                    for h, (off, hs) in enumerate(HS):
                        nc.scalar.activation(
                            ht[:, off : off + hs],
                            psums[f][h][:],
                            GELU,
                            bias=b1t_sb[e][:, col : col + 1],
                        )
                    nc.vector.tensor_mul(ht[:], ht[:], wg_sb[e][:])
                    hts.append(ht)
                w2s = []
                for f in range(FC):
                    w2f = w2_pool.tile(
                        [P, D_MODEL], F32R, tag="w2s", name=f"w2s_{e}_{c}_{f}"
                    )
                    row = (c * FC + f) * P
                    nc.sync.dma_start(w2f[:], w2[e][row : row + P, :])
                    w2s.append(w2f)
                return hts, w2s

            def emit_l2(e, c, hts, w2s):
                for m in range(M2):
                    for h, (off, hs) in enumerate(HS):
                        ps = psum_pool.tile(
                            [P, hs], F32, tag="ps", name=f"ps2_{e}_{c}_{m}_{h}"
                        )
                        for f in range(FC):
                            nc.tensor.matmul(
                                ps[:],
                                w2s[f][:, m * P : (m + 1) * P],
                                hts[f][:, off : off + hs],
                                start=(f == 0),
                                stop=(f == FC - 1),
                            )
                        ysl = y_sb[:, m, off : off + hs]
                        nc.vector.tensor_add(ysl, ysl, ps[:])

            psums_cur = emit_l1(*pairs[0], first=True)

            for m in range(M2):
                nc.vector.tensor_scalar_mul(
                    y_sb[:, m, :], wg_sb[0][:], b2t_sb[0][:, m : m + 1]
                )
                t = ht_pool.tile([P, TOKC], F32, tag="ht", name="ytmp")
                nc.vector.tensor_scalar_mul(
                    t[:], wg_sb[1][:], b2t_sb[1][:, m : m + 1]
                )
                nc.vector.tensor_add(y_sb[:, m, :], y_sb[:, m, :], t[:])

            for i, (e, c) in enumerate(pairs):
                hts, w2s = emit_act(e, c, psums_cur)
                if i + 1 < len(pairs):
                    psums_cur = emit_l1(*pairs[i + 1])
                emit_l2(e, c, hts, w2s)

            yt3 = yt.rearrange("(mo p) t -> p mo t", p=P)
            for m in range(M2):
                nc.sync.dma_start(yt3[:, m, :], y_sb[:, m, :])

    nc.compile()
    return nc


_CACHED = {}


def _get_nc(kind, cap=None):
    key = (kind, cap)
    if key not in _CACHED:
        nc = bacc.Bacc(
            "TRN2",
            target_bir_lowering=False,
            debug=False,
            num_devices=N_CORES,
        )
        if kind == "sparse":
            _CACHED[key] = _build_sparse(nc, cap)
        else:
            _CACHED[key] = _build_dense(nc)
    return _CACHED[key]


def _run(nc, in_maps):
    trace = bool(int(os.environ.get("KERNEL_TRACE", "0")))
    if trace:
        _ensure_ntff_hook()
    res = bass_utils.run_bass_kernel_spmd(
        nc, in_maps, core_ids=list(range(N_CORES)), trace=trace
    )
    if trace:
        kernel.last_exec_time_ns = res.exec_time_ns
        kernel.last_results = res
    return res


def kernel(**inputs):
    x = np.asarray(inputs["x_local"], dtype=np.float32)          # (8192, 2048)
    ids = np.asarray(inputs["top2_exp_id"])                       # (8192, 2)
    tw = np.asarray(inputs["top2_weight"], dtype=np.float32)      # (8192, 2)

    sel = (ids % 2).astype(np.float32)
    wge = [
        (tw * (1.0 - sel)).sum(axis=1).astype(np.float32),        # expert-0 gate
        (tw * sel).sum(axis=1).astype(np.float32),                # expert-1 gate
    ]

    xtb = np.ascontiguousarray(x.T.astype(BF16NP))                # (2048, 8192) bf16

    sparse_shared = {}
    for e in range(2):
        w1b = np.asarray(inputs[f"W1_{e}"], dtype=np.float32).astype(BF16NP)
        # pack into quad-strip blocks [q, c, p, kk, f]: one [P,4,CHUNK]
        # quad per DMA with 4KB contiguous per-partition lines
        sparse_shared[f"w1_{e}"] = np.ascontiguousarray(
            w1b.reshape(KM // 4, 4, P, NCHUNK, CHUNK).transpose(0, 3, 2, 1, 4)
        )
        sparse_shared[f"w2_{e}"] = np.ascontiguousarray(
            np.asarray(inputs[f"W2_{e}"], dtype=np.float32).astype(BF16NP)
        )
        sparse_shared[f"b1t_{e}"] = np.ascontiguousarray(
            np.asarray(inputs[f"b1_{e}"], dtype=np.float32).reshape(D_FF // P, P).T
        )
        sparse_shared[f"b2t_{e}"] = np.ascontiguousarray(
            np.asarray(inputs[f"b2_{e}"], dtype=np.float32).reshape(M2, P).T
        )

    # Globally-balanced gathers: each expert's active set (~75% of all
    # tokens) is split evenly across the 8 cores, so per-core load is
    # |S_e|/8 +- 1 regardless of which core a token "belongs" to.
    # CAP is sized to the observed routing (rounded up to a multiple of
    # 16 so the compiled kernel is reused across calls with similar
    # counts); compile time is per-CAP one-time, off the HW clock.
    glocs = [np.flatnonzero(wge[e] > 0) for e in range(2)]
    caps = tuple(
        max(512, ((-(-len(g) // N_CORES) + 7) // 8) * 8) for g in glocs
    )
    overflow = max(caps) > CAP_MAX

    if not overflow:
        splits = [np.array_split(glocs[e], N_CORES) for e in range(2)]
        in_maps = []
        for c in range(N_CORES):
            m = dict(sparse_shared)
            for e in range(2):
                cap = caps[e]
                loc = splits[e][c]
                cnt = len(loc)
                xgc = np.zeros((D_MODEL, cap), BF16NP)
                xgc[:, :cnt] = xtb[:, loc]
                # partition-major [p, k, t]
                m[f"xg{e}"] = np.ascontiguousarray(
                    xgc.reshape(KM, P, cap).transpose(1, 0, 2)
                )
                wggc = np.zeros((cap,), np.float32)
                wggc[:cnt] = wge[e][loc]
                m[f"wgg{e}"] = np.ascontiguousarray(
                    np.broadcast_to(wggc, (P, cap))
                )
            in_maps.append(m)

        res = _run(_get_nc("sparse", caps), in_maps)

        y = np.zeros((N_LOCAL, D_MODEL), np.float32)
        for c in range(N_CORES):
            for e in range(2):
                loc = splits[e][c]
                cnt = len(loc)
                r = res.results[c][f"yt{e}"]  # (P, M2, cap) bf16
                yd = r.transpose(1, 0, 2).reshape(D_MODEL, caps[e])
                y[loc] += yd.T[:cnt].astype(np.float32)
        return y

    # dense fallback (vanishingly rare: a gather exceeded capacity)
    xt = np.ascontiguousarray(x.T)
    dense_shared = {}
    for e in range(2):
        dense_shared[f"w1_{e}"] = np.ascontiguousarray(
            np.asarray(inputs[f"W1_{e}"], dtype=np.float32)
        )
        dense_shared[f"w2_{e}"] = np.ascontiguousarray(
            np.asarray(inputs[f"W2_{e}"], dtype=np.float32)
        )
        dense_shared[f"b1t_{e}"] = sparse_shared[f"b1t_{e}"]
        dense_shared[f"b2t_{e}"] = sparse_shared[f"b2t_{e}"]
    in_maps = []
    for c in range(N_CORES):
        tok = slice(c * TOKC, (c + 1) * TOKC)
        m = dict(dense_shared)
        m["xt"] = np.ascontiguousarray(xt[:, tok])
        for e in range(2):
            m[f"wg{e}"] = np.ascontiguousarray(
                np.broadcast_to(wge[e][tok], (P, TOKC)).astype(np.float32)
            )
        in_maps.append(m)
    res = _run(_get_nc("dense"), in_maps)
    ytc = np.concatenate([r["yt"] for r in res.results], axis=1)  # (2048, 8192)
    return np.ascontiguousarray(ytc.T)


# revision 59
# speedup vs baseline: 1.7571x; 1.0015x over previous
"""MoE top-2 (2 experts) FFN kernel for TRN2, 8 NeuronCores.

Problem (hardcoded):
  x:   (8192, 2048) f32 tokens
  two expert FFNs: d_model=2048 -> d_ff=8192 (gelu exact) -> 2048
  out[i] = w0[i] * FFN0(x[i]) + w1[i] * FFN1(x[i])
  where w_e[i] = sum of top2_weight[i, s] over slots s with (top2_exp_id[i,s] % 2) == e

Strategy:
  - Host: fold top-2 gating into per-token scalars w0/w1; gather each
    expert's active tokens (those with w_e > 0, ~75% of tokens) with a
    globally balanced split across the 8 cores. Capacity CAP is sized
    per call from the observed routing counts (compile is per-CAP,
    one-time, off the HW clock); dense fallback above CAP_MAX.
  - bf16 weights + activations (fp32 PSUM accumulate, fp32 gelu/y):
    halves HBM traffic and enables FWL so LDWEIGHTS fully hides under
    the 1-cycle/row matmul stream; l2 err ~4e-3 vs the 2e-2 gate.
  - On-core: activations kept transposed ([d_model|d_ff on partitions] x
    [tokens on free dim]) so both matmul layers contract along partitions.
    W1 host-packed into contiguous [P,4,CHUNK] quad-strips (one 512KB DMA
    with 4KB per-partition lines each); xg/yt partition-major.
  - Both experts' gathered xT stay SBUF-resident (~6MB bf16); expert 1's
    tiles prefetch during expert 0 compute -> no stall at the switch.
  - d_ff processed in chunks of 512; layer-2 partials accumulated into an
    SBUF-resident fp32 y; the last chunk's accumulate writes bf16 and
    stores half the bytes. Each weight byte is streamed exactly once.
  - Software-pipelined emission: PE order L1(0),L1(1),L2(0),L1(2),L2(1)...
    so gelu/gate (ACT+DVE) of chunk i overlaps L1(i+1) matmuls. Steady
    state issues matmuls at the N/2.4GHz+2.5ns hardware floor (~96% MFU).
  - Startup: garbage warm-up matmuls flip the HAM clock gate to 8/8 and a
    dummy gelu preloads the ACT table while the first DMAs land; startup
    DMAs are need-ordered across the two HW-DGE queues (sync + scalar),
    keeping the scalar FIFO free ahead of the first gelu (shared DMA ring
    credits would block it and stall the next chunk's PSUM release).
"""

import os

import ml_dtypes
import numpy as np

import concourse.bass as bass
import concourse.mybir as mybir
import concourse.tile as tile
from concourse import bacc
from concourse import bass_utils

BF16NP = ml_dtypes.bfloat16


def _ensure_ntff_hook():
    """This image's `antenv` lacks `axon_hooks`, so boot-time NTFF hook
    install degrades silently and trace=True captures nothing. Register a
    shim module and install the ctypes-driven hook (same as trn_boot)."""
    import sys
    import types

    if "antenv.axon_hooks" in sys.modules:
        return
    mod = types.ModuleType("antenv.axon_hooks")
    mod._hook = None

    def set_axon_ntff_profile_hook(h):
        mod._hook = h

    def get_axon_ntff_profile_hook():
        return mod._hook

    mod.set_axon_ntff_profile_hook = set_axon_ntff_profile_hook
    mod.get_axon_ntff_profile_hook = get_axon_ntff_profile_hook
    sys.modules["antenv.axon_hooks"] = mod
    try:
        from trn_agent_boot.trn_boot import _ntff_profile_via_ctypes

        hook = _ntff_profile_via_ctypes("/opt/axon/libaxon_pjrt.so")
        if hook is not None:
            mod._hook = hook
    except Exception:
        pass


P = 128
D_MODEL = 2048
D_FF = 8192
N_LOCAL = 8192
N_CORES = 8
TOKC = N_LOCAL // N_CORES      # 1024 tokens per core
CAP_MAX = 832                  # per-expert gathered-token capacity limit
                               # (SBUF budget); dense fallback above this.
                               # actual CAP is chosen per call from the
                               # observed routing counts.
KM = D_MODEL // P              # 16 contraction tiles for layer 1
CHUNK = 512                    # d_ff chunk held in PSUM per pass
FC = CHUNK // P                # 4 d_ff tiles per chunk
NCHUNK = D_FF // CHUNK         # 16
M2 = D_MODEL // P              # 16 output d_model tiles

F32 = mybir.dt.float32
F32R = mybir.dt.float32r
BF16 = mybir.dt.bfloat16
GELU = mybir.ActivationFunctionType.Gelu


def _blocks(total):
    """Moving-dim blocks: each <= 512 (PSUM bank limit for f32 output).
    Equal blocks keep per-matmul overhead uniform."""
    n = (total + 511) // 512
    base = total // n
    out = []
    off = 0
    for i in range(n):
        hs = base + (1 if i < total - base * n else 0)
        out.append((off, hs))
        off += hs
    assert off == total and all(hs <= 512 for _, hs in out)
    return out


def _build_sparse(nc, caps):
    """Per-expert gathered tokens (caps[e] per core); expert passes run
    back-to-back; both experts' xT tiles are SBUF-resident."""
    HSs = [_blocks(caps[e]) for e in range(2)]
    CAPM = max(caps)
    # xg host-packed partition-major: [p, k, t] -> per-partition lines of
    # (k1-k0)*cap*2B per group DMA
    xg = [
        nc.dram_tensor(f"xg{e}", (P, KM, caps[e]), BF16,
                       kind="ExternalInput").ap()
        for e in range(2)
    ]
    # w1 host-packed quad-strips: [q, c, p, kk, f] -> a [P, 4, CHUNK] quad
    # is one contiguous 512KB block with 4KB per-partition lines
    w1 = [
        nc.dram_tensor(f"w1_{e}", (KM // 4, NCHUNK, P, 4, CHUNK), BF16,
                       kind="ExternalInput").ap()
        for e in range(2)
    ]
    w2 = [
        nc.dram_tensor(f"w2_{e}", (D_FF, D_MODEL), BF16, kind="ExternalInput").ap()
        for e in range(2)
    ]
    b1t = [
        nc.dram_tensor(f"b1t_{e}", (P, D_FF // P), F32, kind="ExternalInput").ap()
        for e in range(2)
    ]
    b2t = [
        nc.dram_tensor(f"b2t_{e}", (P, M2), F32, kind="ExternalInput").ap()
        for e in range(2)
    ]
    wgg = [
        nc.dram_tensor(f"wgg{e}", (P, caps[e]), F32, kind="ExternalInput").ap()
        for e in range(2)
    ]
    # yt partition-major: [p, m, t]
    yt = [
        nc.dram_tensor(f"yt{e}", (P, M2, caps[e]), BF16,
                       kind="ExternalOutput").ap()
        for e in range(2)
    ]

    with tile.TileContext(nc) as tc:
        with (
            tc.tile_pool(name="const", bufs=1) as const_pool,
            tc.tile_pool(name="w1s", bufs=8) as w1_pool,
            tc.tile_pool(name="w2s", bufs=6) as w2_pool,
            tc.tile_pool(name="ht", bufs=8) as ht_pool,
            tc.tile_pool(name="ps", bufs=8, space="PSUM") as psum_pool,
        ):
            # Both experts' xT k-tiles live in SBUF simultaneously (bf16).
            xt_sb = [
                const_pool.tile([P, KM, caps[e]], BF16, tag=f"xt{e}",
                                name=f"xt_sb{e}")
                for e in range(2)
            ]
            y_sb = const_pool.tile([P, M2, CAPM], F32, tag="y", name="y_sb")
            # final-chunk accumulate lands here in bf16 -> half-size stores
            yb_sb = const_pool.tile([P, M2, CAPM], BF16, tag="yb", name="yb_sb")
            wgg_sb = [
                const_pool.tile([P, caps[e]], F32, tag=f"wgg{e}",
                                name=f"wgg{e}_sb")
                for e in range(2)
            ]
            b1t_sb = [
                const_pool.tile([P, D_FF // P], F32, tag=f"b1t{e}", name=f"b1t{e}_sb")
                for e in range(2)
            ]
            b2t_sb = [
                const_pool.tile([P, M2], F32, tag=f"b2t{e}", name=f"b2t{e}_sb")
                for e in range(2)
            ]

            xg3 = xg
            yt3 = yt

            # chunk schedule: (expert, d_ff tile start fi0, n tiles fc)
            NFI = D_FF // P  # 64
            chunks = [
                (e, c * FC, FC) for e in range(2) for c in range(NCHUNK)
            ]

            # xT k-tile groups: small up front so the first matmuls aren't
            # gated on a big transfer, fat later. Groups MUST be emitted at
            # or before the w1-quad loop that consumes their k range.
            XT_GROUPS = [(0, 1), (1, 2), (2, 4), (4, 8), (8, 12), (12, 16)]

            # warm-up: garbage matmuls while the startup DMAs are in
            # flight, so HAM un-throttles the PE (~3.4us of activity)
            # before the first real matmul
            dummy = const_pool.tile([P, 256], BF16, tag="dummy", name="dummy_sb")
            nc.gpsimd.memset(dummy[:], 0)
            # preload the gelu table (1.3us ACT_TABLE_LOAD) off the
            # critical path, before the first real gelu needs it
            nc.scalar.activation(dummy[:, 128:256], dummy[:, :128], GELU)
            dps = psum_pool.tile([P, 128], F32, tag="ps", name="dummy_ps")

            def warm(n):
                for _ in range(n):
                    nc.tensor.matmul(dps[:], dummy[:, :P], dummy[:, :P],
                                     start=True, stop=True)

            warm(28)

            def emit_aux(e):
                # tiny b1t/b2t on scalar (they release ring credits fast
                # and gelu needs b1t); fat wgg goes on sync separately
                nc.scalar.dma_start(b1t_sb[e][:], b1t[e][:])
                nc.scalar.dma_start(b2t_sb[e][:], b2t[e][:])

            def emit_l1(ci, e, fi0, fc, first=False):
                """PE: layer-1 matmuls for one (expert, chunk)."""
                cb, fo = fi0 // FC, (fi0 % FC) * P
                ncol = fc * P
                HS = HSs[e]
                psums = [
                    [
                        psum_pool.tile(
                            [P, hs], F32, tag="ps", name=f"ps1_{e}_{fi0}_{f}_{h}"
                        )
                        for h, (off, hs) in enumerate(HS)
                    ]
                    for f in range(fc)
                ]
                for q in range(KM // 4):
                    # quad-strip w1 tile: 4 k-strips per DMA enqueue
                    w1s = w1_pool.tile(
                        [P, 4, CHUNK], BF16, tag="w1s", name=f"w1s_{e}_{fi0}_{q}"
                    )
                    if first and q == 0:
                        # sub-loads so the first matmuls gate on 64KB
                        for k0, k1 in ((0, 1), (1, 2), (2, 4)):
                            nc.sync.dma_start(
                                w1s[:, k0:k1, :ncol],
                                w1[e][q, cb, :, k0:k1, fo : fo + ncol],
                            )
                    else:
                        nc.sync.dma_start(
                            w1s[:, :, :ncol], w1[e][q, cb, :, :, fo : fo + ncol]
                        )
                    if first:
                        # aux first: tiny, needed by the first gelu; the
                        # ONLY scalar-queue DMAs before the first gelu
                        # (shared ring credits would block it otherwise)
                        if q == 0:
                            emit_aux(0)
                        # expert-0 xT: the two tiny head groups ride the
                        # scalar queue (their rings clear long before the
                        # first gelu), the fat rest interleave with the w1
                        # quads on sync in need-order
                        for gi in ({0: (0, 1, 2), 1: (3,), 2: (4,), 3: (5,)}[q]):
                            k0, k1 = XT_GROUPS[gi]
                            nc.sync.dma_start(
                                xt_sb[0][:, k0:k1, :], xg3[0][:, k0:k1, :]
                            )
                        if q == 3:
                            nc.sync.dma_start(wgg_sb[0][:], wgg[0][:])
                    if ci == 1 and q == 0:
                        emit_aux(1)
                        nc.sync.dma_start(wgg_sb[1][:], wgg[1][:])
                    if 3 <= ci <= 6 and q == 0:
                        # prefetch expert-1 xT (one 4-tile group per chunk)
                        # well before the expert switch, on the scalar queue
                        k0 = 4 * (ci - 3)
                        nc.scalar.dma_start(
                            xt_sb[1][:, k0 : k0 + 4, :], xg3[1][:, k0 : k0 + 4, :]
                        )
                    for kk in range(4):
                        k = 4 * q + kk
                        for f in range(fc):
                            for h, (off, hs) in enumerate(HS):
                                nc.tensor.matmul(
                                    psums[f][h][:],
                                    w1s[:, kk, f * P : (f + 1) * P],
                                    xt_sb[e][:, k, off : off + hs],
                                    start=(k == 0),
                                    stop=(k == KM - 1),
                                )
                    if ci == 0:
                        # keep HAM warm through the startup DMA trickle:
                        # these run in PE-idle gaps and cost ~0.4us each
                        # batch when data is on time; the q==3 batch also
                        # bridges the chunk0->1 gelu/PSUM-release wait
                        warm(16 if q == 3 else 6)
                return psums

            def emit_act(e, fi0, fc, psums, first=False):
                """ACT+DVE: gelu(+b1), gate scale. Also W2 strip loads
                (scalar queue, enqueued BEFORE the gelu ops so they don't
                wait behind them in the FIFO — except the very first chunk,
                where the startup DMA crunch would exhaust ring credits and
                block the first gelu, stalling the next chunk's PSUM
                release), and (on each expert's first chunk) the gated b2
                y-init."""
                def emit_w2():
                    w2s = []
                    for f in range(fc):
                        w2f = w2_pool.tile(
                            [P, D_MODEL], BF16, tag="w2s",
                            name=f"w2s_{e}_{fi0}_{f}"
                        )
                        row = (fi0 + f) * P
                        nc.scalar.dma_start(w2f[:], w2[e][row : row + P, :])
                        w2s.append(w2f)
                    return w2s

                HS = HSs[e]
                w2s = None if first else emit_w2()
                if fi0 == 0:
                    for m in range(M2):
                        nc.vector.tensor_scalar_mul(
                            y_sb[:, m, : caps[e]], wgg_sb[e][:],
                            b2t_sb[e][:, m : m + 1]
                        )
                hts = []
                for f in range(fc):
                    ht = ht_pool.tile(
                        [P, caps[e]], BF16, tag="ht", name=f"ht_{e}_{fi0}_{f}"
                    )
                    col = fi0 + f
                    for h, (off, hs) in enumerate(HS):
                        nc.scalar.activation(
                            ht[:, off : off + hs],
                            psums[f][h][:],
                            GELU,
                            bias=b1t_sb[e][:, col : col + 1],
                        )
                    nc.vector.tensor_mul(ht[:], ht[:], wgg_sb[e][:])
                    hts.append(ht)
                if w2s is None:
                    w2s = emit_w2()
                return hts, w2s

            def emit_l2(e, fi0, fc, hts, w2s):
                """PE: layer-2 matmuls; DVE: accumulate into y; the last
                chunk's accumulate writes bf16 and stores half the bytes."""
                HS = HSs[e]
                last = fi0 + fc == NFI
                for m in range(M2):
                    for h, (off, hs) in enumerate(HS):
                        ps = psum_pool.tile(
                            [P, hs], F32, tag="ps", name=f"ps2_{e}_{fi0}_{m}_{h}"
                        )
                        for f in range(fc):
                            nc.tensor.matmul(
                                ps[:],
                                w2s[f][:, m * P : (m + 1) * P],
                                hts[f][:, off : off + hs],
                                start=(f == 0),
                                stop=(f == fc - 1),
                            )
                        ysl = y_sb[:, m, off : off + hs]
                        if last:
                            nc.vector.tensor_add(
                                yb_sb[:, m, off : off + hs], ysl, ps[:]
                            )
                        else:
                            nc.vector.tensor_add(ysl, ysl, ps[:])
                    if last:
                        # split the final stores across both HW-DGE queues
                        eng = nc.sync if m % 2 == 0 else nc.scalar
                        eng.dma_start(
                            yt3[e][:, m, :], yb_sb[:, m, : caps[e]]
                        )

            psums_cur = emit_l1(0, *chunks[0], first=True)
            for i, (e, fi0, fc) in enumerate(chunks):
                hts, w2s = emit_act(e, fi0, fc, psums_cur, first=(i == 0))
                if i + 1 < len(chunks):
                    psums_cur = emit_l1(i + 1, *chunks[i + 1])
                emit_l2(e, fi0, fc, hts, w2s)

    nc.compile()
    return nc


def _build_dense(nc):
    """Dense fallback: both experts over all tokens, gate-weighted."""
    HS = [(0, 512), (512, 512)]
    xt = nc.dram_tensor("xt", (D_MODEL, TOKC), F32R, kind="ExternalInput").ap()
    w1 = [
        nc.dram_tensor(f"w1_{e}", (D_MODEL, D_FF), F32R, kind="ExternalInput").ap()
        for e in range(2)
    ]
    w2 = [
        nc.dram_tensor(f"w2_{e}", (D_FF, D_MODEL), F32R, kind="ExternalInput").ap()
        for e in range(2)
    ]
    b1t = [
        nc.dram_tensor(f"b1t_{e}", (P, D_FF // P), F32, kind="ExternalInput").ap()
        for e in range(2)
    ]
    b2t = [
        nc.dram_tensor(f"b2t_{e}", (P, M2), F32, kind="ExternalInput").ap()
        for e in range(2)
    ]
    wg = [
        nc.dram_tensor(f"wg{e}", (P, TOKC), F32, kind="ExternalInput").ap()
        for e in range(2)
    ]
    yt = nc.dram_tensor("yt", (D_MODEL, TOKC), F32, kind="ExternalOutput").ap()

    with tile.TileContext(nc) as tc:
        with (
            tc.tile_pool(name="const", bufs=1) as const_pool,
            tc.tile_pool(name="w1s", bufs=5) as w1_pool,
            tc.tile_pool(name="w2s", bufs=5) as w2_pool,
            tc.tile_pool(name="ht", bufs=5) as ht_pool,
            tc.tile_pool(name="ps", bufs=8, space="PSUM") as psum_pool,
        ):
            xt_sb = const_pool.tile([P, KM, TOKC], F32R, tag="xt", name="xt_sb")
            y_sb = const_pool.tile([P, M2, TOKC], F32, tag="y", name="y_sb")
            wg_sb = [
                const_pool.tile([P, TOKC], F32, tag=f"wg{e}", name=f"wg{e}_sb")
                for e in range(2)
            ]
            b1t_sb = [
                const_pool.tile([P, D_FF // P], F32, tag=f"b1t{e}", name=f"b1t{e}_sb")
                for e in range(2)
            ]
            b2t_sb = [
                const_pool.tile([P, M2], F32, tag=f"b2t{e}", name=f"b2t{e}_sb")
                for e in range(2)
            ]

            xt3 = xt.rearrange("(ko p) t -> p ko t", p=P)
            pairs = [(e, c) for e in range(2) for c in range(NCHUNK)]

            def emit_l1(e, c, first=False):
                psums = [
                    [
                        psum_pool.tile(
                            [P, hs], F32, tag="ps", name=f"ps1_{e}_{c}_{f}_{h}"
                        )
                        for h, (off, hs) in enumerate(HS)
                    ]
                    for f in range(FC)
                ]
                for k in range(KM):
                    if first:
                        nc.sync.dma_start(xt_sb[:, k, :], xt3[:, k, :])
                        if k == 0:
                            for ee in range(2):
                                nc.sync.dma_start(wg_sb[ee][:], wg[ee][:])
                                nc.sync.dma_start(b1t_sb[ee][:], b1t[ee][:])
                                nc.sync.dma_start(b2t_sb[ee][:], b2t[ee][:])
                    w1s = w1_pool.tile(
                        [P, CHUNK], F32R, tag="w1s", name=f"w1s_{e}_{c}_{k}"
                    )
                    nc.sync.dma_start(
                        w1s[:],
                        w1[e][k * P : (k + 1) * P, c * CHUNK : (c + 1) * CHUNK],
                    )
                    for f in range(FC):
                        for h, (off, hs) in enumerate(HS):
                            nc.tensor.matmul(
                                psums[f][h][:],
                                w1s[:, f * P : (f + 1) * P],
                                xt_sb[:, k, off : off + hs],
                                start=(k == 0),
                                stop=(k == KM - 1),
                            )
                return psums

            def emit_act(e, c, psums):
                hts = []
                for f in range(FC):
                    ht = ht_pool.tile(
                        [P, TOKC], F32R, tag="ht", name=f"ht_{e}_{c}_{f}"
                    )
                    col = c * FC + f
                    for h, (off, hs) in enumerate(HS):
                        nc.scalar.activation(
                            ht[:, off : off + hs],
                            psums[f][h][:],
                            GELU,
                            bias=b1t_sb[e][:, col : col + 1],
                        )
                    nc.vector.tensor_mul(ht[:], ht[:], wg_sb[e][:])
                    hts.append(ht)
                w2s = []
                for f in range(FC):
                    w2f = w2_pool.tile(
                        [P, D_MODEL], F32R, tag="w2s", name=f"w2s_{e}_{c}_{f}"
                    )
                    row = (c * FC + f) * P
                    nc.sync.dma_start(w2f[:], w2[e][row : row + P, :])
                    w2s.append(w2f)
                return hts, w2s

            def emit_l2(e, c, hts, w2s):
                for m in range(M2):
                    for h, (off, hs) in enumerate(HS):
                        ps = psum_pool.tile(
                            [P, hs], F32, tag="ps", name=f"ps2_{e}_{c}_{m}_{h}"
                        )
                        for f in range(FC):
                            nc.tensor.matmul(
                                ps[:],
                                w2s[f][:, m * P : (m + 1) * P],
                                hts[f][:, off : off + hs],
                                start=(f == 0),
                                stop=(f == FC - 1),
                            )
                        ysl = y_sb[:, m, off : off + hs]
                        nc.vector.tensor_add(ysl, ysl, ps[:])

            psums_cur = emit_l1(*pairs[0], first=True)

            for m in range(M2):
                nc.vector.tensor_scalar_mul(
                    y_sb[:, m, :], wg_sb[0][:], b2t_sb[0][:, m : m + 1]
                )
                t = ht_pool.tile([P, TOKC], F32, tag="ht", name="ytmp")
                nc.vector.tensor_scalar_mul(
                    t[:], wg_sb[1][:], b2t_sb[1][:, m : m + 1]
                )
                nc.vector.tensor_add(y_sb[:, m, :], y_sb[:, m, :], t[:])

            for i, (e, c) in enumerate(pairs):
                hts, w2s = emit_act(e, c, psums_cur)
                if i + 1 < len(pairs):
                    psums_cur = emit_l1(*pairs[i + 1])
                emit_l2(e, c, hts, w2s)

            yt3 = yt.rearrange("(mo p) t -> p mo t", p=P)
            for m in range(M2):
                nc.sync.dma_start(yt3[:, m, :], y_sb[:, m, :])

    nc.compile()
    return nc


_CACHED = {}


def _get_nc(kind, cap=None):
    key = (kind, cap)
    if key not in _CACHED:
        nc = bacc.Bacc(
            "TRN2",
            target_bir_lowering=False,
            debug=False,
            num_devices=N_CORES,
        )
        if kind == "sparse":
            _CACHED[key] = _build_sparse(nc, cap)
        else:
            _CACHED[key] = _build_dense(nc)
    return _CACHED[key]


def _run(nc, in_maps):
    trace = bool(int(os.environ.get("KERNEL_TRACE", "0")))
    if trace:
        _ensure_ntff_hook()
    res = bass_utils.run_bass_kernel_spmd(
        nc, in_maps, core_ids=list(range(N_CORES)), trace=trace
    )
    if trace:
        kernel.last_exec_time_ns = res.exec_time_ns
        kernel.last_results = res
    return res


def kernel(**inputs):
    x = np.asarray(inputs["x_local"], dtype=np.float32)          # (8192, 2048)
    ids = np.asarray(inputs["top2_exp_id"])                       # (8192, 2)
    tw = np.asarray(inputs["top2_weight"], dtype=np.float32)      # (8192, 2)

    sel = (ids % 2).astype(np.float32)
    wge = [
        (tw * (1.0 - sel)).sum(axis=1).astype(np.float32),        # expert-0 gate
        (tw * sel).sum(axis=1).astype(np.float32),                # expert-1 gate
    ]

    xtb = np.ascontiguousarray(x.T.astype(BF16NP))                # (2048, 8192) bf16

    sparse_shared = {}
    for e in range(2):
        w1b = np.asarray(inputs[f"W1_{e}"], dtype=np.float32).astype(BF16NP)
        # pack into quad-strip blocks [q, c, p, kk, f]: one [P,4,CHUNK]
        # quad per DMA with 4KB contiguous per-partition lines
        sparse_shared[f"w1_{e}"] = np.ascontiguousarray(
            w1b.reshape(KM // 4, 4, P, NCHUNK, CHUNK).transpose(0, 3, 2, 1, 4)
        )
        sparse_shared[f"w2_{e}"] = np.ascontiguousarray(
            np.asarray(inputs[f"W2_{e}"], dtype=np.float32).astype(BF16NP)
        )
        sparse_shared[f"b1t_{e}"] = np.ascontiguousarray(
            np.asarray(inputs[f"b1_{e}"], dtype=np.float32).reshape(D_FF // P, P).T
        )
        sparse_shared[f"b2t_{e}"] = np.ascontiguousarray(
            np.asarray(inputs[f"b2_{e}"], dtype=np.float32).reshape(M2, P).T
        )

    # Globally-balanced gathers: each expert's active set (~75% of all
    # tokens) is split evenly across the 8 cores, so per-core load is
    # |S_e|/8 +- 1 regardless of which core a token "belongs" to.
    # CAP is sized to the observed routing (rounded up to a multiple of
    # 16 so the compiled kernel is reused across calls with similar
    # counts); compile time is per-CAP one-time, off the HW clock.
    glocs = [np.flatnonzero(wge[e] > 0) for e in range(2)]
    caps = tuple(
        max(512, ((-(-len(g) // N_CORES) + 7) // 8) * 8) for g in glocs
    )
    overflow = max(caps) > CAP_MAX

    if not overflow:
        splits = [np.array_split(glocs[e], N_CORES) for e in range(2)]
        in_maps = []
        for c in range(N_CORES):
            m = dict(sparse_shared)
            for e in range(2):
                cap = caps[e]
                loc = splits[e][c]
                cnt = len(loc)
                xgc = np.zeros((D_MODEL, cap), BF16NP)
                xgc[:, :cnt] = xtb[:, loc]
                # partition-major [p, k, t]
                m[f"xg{e}"] = np.ascontiguousarray(
                    xgc.reshape(KM, P, cap).transpose(1, 0, 2)
                )
                wggc = np.zeros((cap,), np.float32)
                wggc[:cnt] = wge[e][loc]
                m[f"wgg{e}"] = np.ascontiguousarray(
                    np.broadcast_to(wggc, (P, cap))
                )
            in_maps.append(m)

        res = _run(_get_nc("sparse", caps), in_maps)

        y = np.zeros((N_LOCAL, D_MODEL), np.float32)
        for c in range(N_CORES):
            for e in range(2):
                loc = splits[e][c]
                cnt = len(loc)
                r = res.results[c][f"yt{e}"]  # (P, M2, cap) bf16
                yd = r.transpose(1, 0, 2).reshape(D_MODEL, caps[e])
                y[loc] += yd.T[:cnt].astype(np.float32)
        return y

    # dense fallback (vanishingly rare: a gather exceeded capacity)
    xt = np.ascontiguousarray(x.T)
    dense_shared = {}
    for e in range(2):
        dense_shared[f"w1_{e}"] = np.ascontiguousarray(
            np.asarray(inputs[f"W1_{e}"], dtype=np.float32)
        )
        dense_shared[f"w2_{e}"] = np.ascontiguousarray(
            np.asarray(inputs[f"W2_{e}"], dtype=np.float32)
        )
        dense_shared[f"b1t_{e}"] = sparse_shared[f"b1t_{e}"]
        dense_shared[f"b2t_{e}"] = sparse_shared[f"b2t_{e}"]
    in_maps = []
    for c in range(N_CORES):
        tok = slice(c * TOKC, (c + 1) * TOKC)
        m = dict(dense_shared)
        m["xt"] = np.ascontiguousarray(xt[:, tok])
        for e in range(2):
            m[f"wg{e}"] = np.ascontiguousarray(
                np.broadcast_to(wge[e][tok], (P, TOKC)).astype(np.float32)
            )
        in_maps.append(m)
    res = _run(_get_nc("dense"), in_maps)
    ytc = np.concatenate([r["yt"] for r in res.results], axis=1)  # (2048, 8192)
    return np.ascontiguousarray(ytc.T)


# revision 60
# speedup vs baseline: 1.7577x; 1.0004x over previous
"""MoE top-2 (2 experts) FFN kernel for TRN2, 8 NeuronCores.

Problem (hardcoded):
  x:   (8192, 2048) f32 tokens
  two expert FFNs: d_model=2048 -> d_ff=8192 (gelu exact) -> 2048
  out[i] = w0[i] * FFN0(x[i]) + w1[i] * FFN1(x[i])
  where w_e[i] = sum of top2_weight[i, s] over slots s with (top2_exp_id[i,s] % 2) == e

Strategy:
  - Host: fold top-2 gating into per-token scalars w0/w1; gather each
    expert's active tokens (those with w_e > 0, ~75% of tokens) with a
    globally balanced split across the 8 cores. Capacity CAP is sized
    per call from the observed routing counts (compile is per-CAP,
    one-time, off the HW clock); dense fallback above CAP_MAX.
  - bf16 weights + activations (fp32 PSUM accumulate, fp32 gelu/y):
    halves HBM traffic and enables FWL so LDWEIGHTS fully hides under
    the 1-cycle/row matmul stream; l2 err ~4e-3 vs the 2e-2 gate.
  - On-core: activations kept transposed ([d_model|d_ff on partitions] x
    [tokens on free dim]) so both matmul layers contract along partitions.
    W1 host-packed into contiguous [P,4,CHUNK] quad-strips (one 512KB DMA
    with 4KB per-partition lines each); xg/yt partition-major.
  - Both experts' gathered xT stay SBUF-resident (~6MB bf16); expert 1's
    tiles prefetch during expert 0 compute -> no stall at the switch.
  - d_ff processed in chunks of 512; layer-2 partials accumulated into an
    SBUF-resident fp32 y; the last chunk's accumulate writes bf16 and
    stores half the bytes. Each weight byte is streamed exactly once.
  - Software-pipelined emission: PE order L1(0),L1(1),L2(0),L1(2),L2(1)...
    so gelu/gate (ACT+DVE) of chunk i overlaps L1(i+1) matmuls. Steady
    state issues matmuls at the N/2.4GHz+2.5ns hardware floor (~96% MFU).
  - Startup: garbage warm-up matmuls flip the HAM clock gate to 8/8 and a
    dummy gelu preloads the ACT table while the first DMAs land; startup
    DMAs are need-ordered across the two HW-DGE queues (sync + scalar),
    keeping the scalar FIFO free ahead of the first gelu (shared DMA ring
    credits would block it and stall the next chunk's PSUM release).
"""

import os

import ml_dtypes
import numpy as np

import concourse.bass as bass
import concourse.mybir as mybir
import concourse.tile as tile
from concourse import bacc
from concourse import bass_utils

BF16NP = ml_dtypes.bfloat16


def _ensure_ntff_hook():
    """This image's `antenv` lacks `axon_hooks`, so boot-time NTFF hook
    install degrades silently and trace=True captures nothing. Register a
    shim module and install the ctypes-driven hook (same as trn_boot)."""
    import sys
    import types

    if "antenv.axon_hooks" in sys.modules:
        return
    mod = types.ModuleType("antenv.axon_hooks")
    mod._hook = None

    def set_axon_ntff_profile_hook(h):
        mod._hook = h

    def get_axon_ntff_profile_hook():
        return mod._hook

    mod.set_axon_ntff_profile_hook = set_axon_ntff_profile_hook
    mod.get_axon_ntff_profile_hook = get_axon_ntff_profile_hook
    sys.modules["antenv.axon_hooks"] = mod
    try:
        from trn_agent_boot.trn_boot import _ntff_profile_via_ctypes

        hook = _ntff_profile_via_ctypes("/opt/axon/libaxon_pjrt.so")
        if hook is not None:
            mod._hook = hook
    except Exception:
        pass


P = 128
D_MODEL = 2048
D_FF = 8192
N_LOCAL = 8192
N_CORES = 8
TOKC = N_LOCAL // N_CORES      # 1024 tokens per core
CAP_MAX = 832                  # per-expert gathered-token capacity limit
                               # (SBUF budget); dense fallback above this.
                               # actual CAP is chosen per call from the
                               # observed routing counts.
KM = D_MODEL // P              # 16 contraction tiles for layer 1
CHUNK = 512                    # d_ff chunk held in PSUM per pass
FC = CHUNK // P                # 4 d_ff tiles per chunk
NCHUNK = D_FF // CHUNK         # 16
M2 = D_MODEL // P              # 16 output d_model tiles

F32 = mybir.dt.float32
F32R = mybir.dt.float32r
BF16 = mybir.dt.bfloat16
GELU = mybir.ActivationFunctionType.Gelu


def _blocks(total):
    """Moving-dim blocks: each <= 512 (PSUM bank limit for f32 output).
    Equal blocks keep per-matmul overhead uniform."""
    n = (total + 511) // 512
    base = total // n
    out = []
    off = 0
    for i in range(n):
        hs = base + (1 if i < total - base * n else 0)
        out.append((off, hs))
        off += hs
    assert off == total and all(hs <= 512 for _, hs in out)
    return out


def _build_sparse(nc, caps):
    """Per-expert gathered tokens (caps[e] per core); expert passes run
    back-to-back; both experts' xT tiles are SBUF-resident."""
    HSs = [_blocks(caps[e]) for e in range(2)]
    CAPM = max(caps)
    # xg host-packed partition-major: [p, k, t] -> per-partition lines of
    # (k1-k0)*cap*2B per group DMA
    xg = [
        nc.dram_tensor(f"xg{e}", (P, KM, caps[e]), BF16,
                       kind="ExternalInput").ap()
        for e in range(2)
    ]
    # w1 host-packed quad-strips: [q, c, p, kk, f] -> a [P, 4, CHUNK] quad
    # is one contiguous 512KB block with 4KB per-partition lines
    w1 = [
        nc.dram_tensor(f"w1_{e}", (KM // 4, NCHUNK, P, 4, CHUNK), BF16,
                       kind="ExternalInput").ap()
        for e in range(2)
    ]
    w2 = [
        nc.dram_tensor(f"w2_{e}", (D_FF, D_MODEL), BF16, kind="ExternalInput").ap()
        for e in range(2)
    ]
    b1t = [
        nc.dram_tensor(f"b1t_{e}", (P, D_FF // P), F32, kind="ExternalInput").ap()
        for e in range(2)
    ]
    b2t = [
        nc.dram_tensor(f"b2t_{e}", (P, M2), F32, kind="ExternalInput").ap()
        for e in range(2)
    ]
    wgg = [
        nc.dram_tensor(f"wgg{e}", (P, caps[e]), F32, kind="ExternalInput").ap()
        for e in range(2)
    ]
    # yt partition-major: [p, m, t]
    yt = [
        nc.dram_tensor(f"yt{e}", (P, M2, caps[e]), BF16,
                       kind="ExternalOutput").ap()
        for e in range(2)
    ]

    with tile.TileContext(nc) as tc:
        with (
            tc.tile_pool(name="const", bufs=1) as const_pool,
            tc.tile_pool(name="w1s", bufs=8) as w1_pool,
            tc.tile_pool(name="w2s", bufs=6) as w2_pool,
            tc.tile_pool(name="ht", bufs=8) as ht_pool,
            tc.tile_pool(name="ps", bufs=8, space="PSUM") as psum_pool,
        ):
            # Both experts' xT k-tiles live in SBUF simultaneously (bf16).
            xt_sb = [
                const_pool.tile([P, KM, caps[e]], BF16, tag=f"xt{e}",
                                name=f"xt_sb{e}")
                for e in range(2)
            ]
            y_sb = const_pool.tile([P, M2, CAPM], F32, tag="y", name="y_sb")
            # final-chunk accumulate lands here in bf16 -> half-size stores
            yb_sb = const_pool.tile([P, M2, CAPM], BF16, tag="yb", name="yb_sb")
            wgg_sb = [
                const_pool.tile([P, caps[e]], F32, tag=f"wgg{e}",
                                name=f"wgg{e}_sb")
                for e in range(2)
            ]
            b1t_sb = [
                const_pool.tile([P, D_FF // P], F32, tag=f"b1t{e}", name=f"b1t{e}_sb")
                for e in range(2)
            ]
            b2t_sb = [
                const_pool.tile([P, M2], F32, tag=f"b2t{e}", name=f"b2t{e}_sb")
                for e in range(2)
            ]

            xg3 = xg
            yt3 = yt

            # chunk schedule: (expert, d_ff tile start fi0, n tiles fc)
            NFI = D_FF // P  # 64
            chunks = [
                (e, c * FC, FC) for e in range(2) for c in range(NCHUNK)
            ]

            # xT k-tile groups: small up front so the first matmuls aren't
            # gated on a big transfer, fat later. Groups MUST be emitted at
            # or before the w1-quad loop that consumes their k range.
            XT_GROUPS = [(0, 1), (1, 2), (2, 4), (4, 8), (8, 12), (12, 16)]

            # warm-up: garbage matmuls while the startup DMAs are in
            # flight, so HAM un-throttles the PE (~3.4us of activity)
            # before the first real matmul
            dummy = const_pool.tile([P, 256], BF16, tag="dummy", name="dummy_sb")
            nc.gpsimd.memset(dummy[:], 0)
            # preload the gelu table (1.3us ACT_TABLE_LOAD) off the
            # critical path, before the first real gelu needs it
            nc.scalar.activation(dummy[:, 128:256], dummy[:, :128], GELU)
            dps = psum_pool.tile([P, 128], F32, tag="ps", name="dummy_ps")

            def warm(n):
                for _ in range(n):
                    nc.tensor.matmul(dps[:], dummy[:, :P], dummy[:, :P],
                                     start=True, stop=True)

            warm(28)

            def emit_aux(e):
                # tiny b1t/b2t on scalar (they release ring credits fast
                # and gelu needs b1t); fat wgg goes on sync separately
                nc.scalar.dma_start(b1t_sb[e][:], b1t[e][:])
                nc.scalar.dma_start(b2t_sb[e][:], b2t[e][:])

            def emit_l1(ci, e, fi0, fc, first=False):
                """PE: layer-1 matmuls for one (expert, chunk)."""
                cb, fo = fi0 // FC, (fi0 % FC) * P
                ncol = fc * P
                HS = HSs[e]
                psums = [
                    [
                        psum_pool.tile(
                            [P, hs], F32, tag="ps", name=f"ps1_{e}_{fi0}_{f}_{h}"
                        )
                        for h, (off, hs) in enumerate(HS)
                    ]
                    for f in range(fc)
                ]
                for q in range(KM // 4):
                    # quad-strip w1 tile: 4 k-strips per DMA enqueue
                    w1s = w1_pool.tile(
                        [P, 4, CHUNK], BF16, tag="w1s", name=f"w1s_{e}_{fi0}_{q}"
                    )
                    if first and q == 0:
                        # sub-loads so the first matmuls gate on 64KB
                        for k0, k1 in ((0, 1), (1, 2), (2, 4)):
                            nc.sync.dma_start(
                                w1s[:, k0:k1, :ncol],
                                w1[e][q, cb, :, k0:k1, fo : fo + ncol],
                            )
                    else:
                        nc.sync.dma_start(
                            w1s[:, :, :ncol], w1[e][q, cb, :, :, fo : fo + ncol]
                        )
                    if first:
                        # aux first: tiny, needed by the first gelu; the
                        # ONLY scalar-queue DMAs before the first gelu
                        # (shared ring credits would block it otherwise)
                        if q == 0:
                            emit_aux(0)
                        # expert-0 xT: the two tiny head groups ride the
                        # scalar queue (their rings clear long before the
                        # first gelu), the fat rest interleave with the w1
                        # quads on sync in need-order
                        for gi in ({0: (0, 1, 2), 1: (3,), 2: (4,), 3: (5,)}[q]):
                            k0, k1 = XT_GROUPS[gi]
                            nc.sync.dma_start(
                                xt_sb[0][:, k0:k1, :], xg3[0][:, k0:k1, :]
                            )
                        if q == 3:
                            nc.sync.dma_start(wgg_sb[0][:], wgg[0][:])
                    if ci == 1 and q == 0:
                        emit_aux(1)
                        nc.sync.dma_start(wgg_sb[1][:], wgg[1][:])
                    if 3 <= ci <= 6 and q == 0:
                        # prefetch expert-1 xT (one 4-tile group per chunk)
                        # well before the expert switch, on the scalar queue
                        k0 = 4 * (ci - 3)
                        nc.scalar.dma_start(
                            xt_sb[1][:, k0 : k0 + 4, :], xg3[1][:, k0 : k0 + 4, :]
                        )
                    for kk in range(4):
                        k = 4 * q + kk
                        for f in range(fc):
                            for h, (off, hs) in enumerate(HS):
                                nc.tensor.matmul(
                                    psums[f][h][:],
                                    w1s[:, kk, f * P : (f + 1) * P],
                                    xt_sb[e][:, k, off : off + hs],
                                    start=(k == 0),
                                    stop=(k == KM - 1),
                                )
                    if ci == 0:
                        # keep HAM warm through the startup DMA trickle:
                        # these run in PE-idle gaps and cost ~0.4us each
                        # batch when data is on time; the q==3 batch also
                        # bridges the chunk0->1 gelu/PSUM-release wait
                        warm(16 if q == 3 else 6)
                return psums

            def emit_act(e, fi0, fc, psums, first=False):
                """ACT+DVE: gelu(+b1), gate scale. Also W2 strip loads
                (scalar queue, enqueued BEFORE the gelu ops so they don't
                wait behind them in the FIFO — except the very first chunk,
                where the startup DMA crunch would exhaust ring credits and
                block the first gelu, stalling the next chunk's PSUM
                release), and (on each expert's first chunk) the gated b2
                y-init."""
                def emit_w2():
                    w2s = []
                    for f in range(fc):
                        w2f = w2_pool.tile(
                            [P, D_MODEL], BF16, tag="w2s",
                            name=f"w2s_{e}_{fi0}_{f}"
                        )
                        row = (fi0 + f) * P
                        nc.scalar.dma_start(w2f[:], w2[e][row : row + P, :])
                        w2s.append(w2f)
                    return w2s

                HS = HSs[e]
                w2s = None if first else emit_w2()
                if fi0 == 0:
                    for m in range(M2):
                        nc.vector.tensor_scalar_mul(
                            y_sb[:, m, : caps[e]], wgg_sb[e][:],
                            b2t_sb[e][:, m : m + 1]
                        )
                hts = []
                for f in range(fc):
                    ht = ht_pool.tile(
                        [P, caps[e]], BF16, tag="ht", name=f"ht_{e}_{fi0}_{f}"
                    )
                    col = fi0 + f
                    for h, (off, hs) in enumerate(HS):
                        nc.scalar.activation(
                            ht[:, off : off + hs],
                            psums[f][h][:],
                            GELU,
                            bias=b1t_sb[e][:, col : col + 1],
                        )
                    # expert 1's last two chunks have no L1 matmuls left
                    # to cover DVE; gpsimd (idle) takes their gate-muls so
                    # the y-accumulates keep pace with the PE
                    veng = nc.gpsimd if (e == 1 and fi0 >= NFI - 2 * FC) \
                        else nc.vector
                    veng.tensor_mul(ht[:], ht[:], wgg_sb[e][:])
                    hts.append(ht)
                if w2s is None:
                    w2s = emit_w2()
                return hts, w2s

            def emit_l2(e, fi0, fc, hts, w2s):
                """PE: layer-2 matmuls; DVE: accumulate into y; the last
                chunk's accumulate writes bf16 and stores half the bytes."""
                HS = HSs[e]
                last = fi0 + fc == NFI
                for m in range(M2):
                    for h, (off, hs) in enumerate(HS):
                        ps = psum_pool.tile(
                            [P, hs], F32, tag="ps", name=f"ps2_{e}_{fi0}_{m}_{h}"
                        )
                        for f in range(fc):
                            nc.tensor.matmul(
                                ps[:],
                                w2s[f][:, m * P : (m + 1) * P],
                                hts[f][:, off : off + hs],
                                start=(f == 0),
                                stop=(f == fc - 1),
                            )
                        ysl = y_sb[:, m, off : off + hs]
                        if last:
                            nc.vector.tensor_add(
                                yb_sb[:, m, off : off + hs], ysl, ps[:]
                            )
                        else:
                            nc.vector.tensor_add(ysl, ysl, ps[:])
                    if last:
                        # split the final stores across both HW-DGE queues
                        eng = nc.sync if m % 2 == 0 else nc.scalar
                        eng.dma_start(
                            yt3[e][:, m, :], yb_sb[:, m, : caps[e]]
                        )

            psums_cur = emit_l1(0, *chunks[0], first=True)
            for i, (e, fi0, fc) in enumerate(chunks):
                hts, w2s = emit_act(e, fi0, fc, psums_cur, first=(i == 0))
                if i + 1 < len(chunks):
                    psums_cur = emit_l1(i + 1, *chunks[i + 1])
                emit_l2(e, fi0, fc, hts, w2s)

    nc.compile()
    return nc


def _build_dense(nc):
    """Dense fallback: both experts over all tokens, gate-weighted."""
    HS = [(0, 512), (512, 512)]
    xt = nc.dram_tensor("xt", (D_MODEL, TOKC), F32R, kind="ExternalInput").ap()
    w1 = [
        nc.dram_tensor(f"w1_{e}", (D_MODEL, D_FF), F32R, kind="ExternalInput").ap()
        for e in range(2)
    ]
    w2 = [
        nc.dram_tensor(f"w2_{e}", (D_FF, D_MODEL), F32R, kind="ExternalInput").ap()
        for e in range(2)
    ]
    b1t = [
        nc.dram_tensor(f"b1t_{e}", (P, D_FF // P), F32, kind="ExternalInput").ap()
        for e in range(2)
    ]
    b2t = [
        nc.dram_tensor(f"b2t_{e}", (P, M2), F32, kind="ExternalInput").ap()
        for e in range(2)
    ]
    wg = [
        nc.dram_tensor(f"wg{e}", (P, TOKC), F32, kind="ExternalInput").ap()
        for e in range(2)
    ]
    yt = nc.dram_tensor("yt", (D_MODEL, TOKC), F32, kind="ExternalOutput").ap()

    with tile.TileContext(nc) as tc:
        with (
            tc.tile_pool(name="const", bufs=1) as const_pool,
            tc.tile_pool(name="w1s", bufs=5) as w1_pool,
            tc.tile_pool(name="w2s", bufs=5) as w2_pool,
            tc.tile_pool(name="ht", bufs=5) as ht_pool,
            tc.tile_pool(name="ps", bufs=8, space="PSUM") as psum_pool,
        ):
            xt_sb = const_pool.tile([P, KM, TOKC], F32R, tag="xt", name="xt_sb")
            y_sb = const_pool.tile([P, M2, TOKC], F32, tag="y", name="y_sb")
            wg_sb = [
                const_pool.tile([P, TOKC], F32, tag=f"wg{e}", name=f"wg{e}_sb")
                for e in range(2)
            ]
            b1t_sb = [
                const_pool.tile([P, D_FF // P], F32, tag=f"b1t{e}", name=f"b1t{e}_sb")
                for e in range(2)
            ]
            b2t_sb = [
                const_pool.tile([P, M2], F32, tag=f"b2t{e}", name=f"b2t{e}_sb")
                for e in range(2)
            ]

            xt3 = xt.rearrange("(ko p) t -> p ko t", p=P)
            pairs = [(e, c) for e in range(2) for c in range(NCHUNK)]

            def emit_l1(e, c, first=False):
                psums = [
                    [
                        psum_pool.tile(
                            [P, hs], F32, tag="ps", name=f"ps1_{e}_{c}_{f}_{h}"
                        )
                        for h, (off, hs) in enumerate(HS)
                    ]
                    for f in range(FC)
                ]
                for k in range(KM):
                    if first:
                        nc.sync.dma_start(xt_sb[:, k, :], xt3[:, k, :])
                        if k == 0:
                            for ee in range(2):
                                nc.sync.dma_start(wg_sb[ee][:], wg[ee][:])
                                nc.sync.dma_start(b1t_sb[ee][:], b1t[ee][:])
                                nc.sync.dma_start(b2t_sb[ee][:], b2t[ee][:])
                    w1s = w1_pool.tile(
                        [P, CHUNK], F32R, tag="w1s", name=f"w1s_{e}_{c}_{k}"
                    )
                    nc.sync.dma_start(
                        w1s[:],
                        w1[e][k * P : (k + 1) * P, c * CHUNK : (c + 1) * CHUNK],
                    )
                    for f in range(FC):
                        for h, (off, hs) in enumerate(HS):
                            nc.tensor.matmul(
                                psums[f][h][:],
                                w1s[:, f * P : (f + 1) * P],
                                xt_sb[:, k, off : off + hs],
                                start=(k == 0),
                                stop=(k == KM - 1),
                            )
                return psums

            def emit_act(e, c, psums):
                hts = []
                for f in range(FC):
                    ht = ht_pool.tile(
                        [P, TOKC], F32R, tag="ht", name=f"ht_{e}_{c}_{f}"
                    )
                    col = c * FC + f
                    for h, (off, hs) in enumerate(HS):
                        nc.scalar.activation(
                            ht[:, off : off + hs],
                            psums[f][h][:],
                            GELU,
                            bias=b1t_sb[e][:, col : col + 1],
                        )
                    nc.vector.tensor_mul(ht[:], ht[:], wg_sb[e][:])
                    hts.append(ht)
                w2s = []
                for f in range(FC):
                    w2f = w2_pool.tile(
                        [P, D_MODEL], F32R, tag="w2s", name=f"w2s_{e}_{c}_{f}"
                    )
                    row = (c * FC + f) * P
                    nc.sync.dma_start(w2f[:], w2[e][row : row + P, :])
                    w2s.append(w2f)
                return hts, w2s

            def emit_l2(e, c, hts, w2s):
                for m in range(M2):
                    for h, (off, hs) in enumerate(HS):
                        ps = psum_pool.tile(
                            [P, hs], F32, tag="ps", name=f"ps2_{e}_{c}_{m}_{h}"
                        )
                        for f in range(FC):
                            nc.tensor.matmul(
                                ps[:],
                                w2s[f][:, m * P : (m + 1) * P],
                                hts[f][:, off : off + hs],
                                start=(f == 0),
                                stop=(f == FC - 1),
                            )
                        ysl = y_sb[:, m, off : off + hs]
                        nc.vector.tensor_add(ysl, ysl, ps[:])

            psums_cur = emit_l1(*pairs[0], first=True)

            for m in range(M2):
                nc.vector.tensor_scalar_mul(
                    y_sb[:, m, :], wg_sb[0][:], b2t_sb[0][:, m : m + 1]
                )
                t = ht_pool.tile([P, TOKC], F32, tag="ht", name="ytmp")
                nc.vector.tensor_scalar_mul(
                    t[:], wg_sb[1][:], b2t_sb[1][:, m : m + 1]
                )
                nc.vector.tensor_add(y_sb[:, m, :], y_sb[:, m, :], t[:])

            for i, (e, c) in enumerate(pairs):
                hts, w2s = emit_act(e, c, psums_cur)
                if i + 1 < len(pairs):
                    psums_cur = emit_l1(*pairs[i + 1])
                emit_l2(e, c, hts, w2s)

            yt3 = yt.rearrange("(mo p) t -> p mo t", p=P)
            for m in range(M2):
                nc.sync.dma_start(yt3[:, m, :], y_sb[:, m, :])

    nc.compile()
    return nc


_CACHED = {}


def _get_nc(kind, cap=None):
    key = (kind, cap)
    if key not in _CACHED:
        nc = bacc.Bacc(
            "TRN2",
            target_bir_lowering=False,
            debug=False,
            num_devices=N_CORES,
        )
        if kind == "sparse":
            _CACHED[key] = _build_sparse(nc, cap)
        else:
            _CACHED[key] = _build_dense(nc)
    return _CACHED[key]


def _run(nc, in_maps):
    trace = bool(int(os.environ.get("KERNEL_TRACE", "0")))
    if trace:
        _ensure_ntff_hook()
    res = bass_utils.run_bass_kernel_spmd(
        nc, in_maps, core_ids=list(range(N_CORES)), trace=trace
    )
    if trace:
        kernel.last_exec_time_ns = res.exec_time_ns
        kernel.last_results = res
    return res


def kernel(**inputs):
    x = np.asarray(inputs["x_local"], dtype=np.float32)          # (8192, 2048)
    ids = np.asarray(inputs["top2_exp_id"])                       # (8192, 2)
    tw = np.asarray(inputs["top2_weight"], dtype=np.float32)      # (8192, 2)

    sel = (ids % 2).astype(np.float32)
    wge = [
        (tw * (1.0 - sel)).sum(axis=1).astype(np.float32),        # expert-0 gate
        (tw * sel).sum(axis=1).astype(np.float32),                # expert-1 gate
    ]

    xtb = np.ascontiguousarray(x.T.astype(BF16NP))                # (2048, 8192) bf16

    sparse_shared = {}
    for e in range(2):
        w1b = np.asarray(inputs[f"W1_{e}"], dtype=np.float32).astype(BF16NP)
        # pack into quad-strip blocks [q, c, p, kk, f]: one [P,4,CHUNK]
        # quad per DMA with 4KB contiguous per-partition lines
        sparse_shared[f"w1_{e}"] = np.ascontiguousarray(
            w1b.reshape(KM // 4, 4, P, NCHUNK, CHUNK).transpose(0, 3, 2, 1, 4)
        )
        sparse_shared[f"w2_{e}"] = np.ascontiguousarray(
            np.asarray(inputs[f"W2_{e}"], dtype=np.float32).astype(BF16NP)
        )
        sparse_shared[f"b1t_{e}"] = np.ascontiguousarray(
            np.asarray(inputs[f"b1_{e}"], dtype=np.float32).reshape(D_FF // P, P).T
        )
        sparse_shared[f"b2t_{e}"] = np.ascontiguousarray(
            np.asarray(inputs[f"b2_{e}"], dtype=np.float32).reshape(M2, P).T
        )

    # Globally-balanced gathers: each expert's active set (~75% of all
    # tokens) is split evenly across the 8 cores, so per-core load is
    # |S_e|/8 +- 1 regardless of which core a token "belongs" to.
    # CAP is sized to the observed routing (rounded up to a multiple of
    # 16 so the compiled kernel is reused across calls with similar
    # counts); compile time is per-CAP one-time, off the HW clock.
    glocs = [np.flatnonzero(wge[e] > 0) for e in range(2)]
    caps = tuple(
        max(512, ((-(-len(g) // N_CORES) + 7) // 8) * 8) for g in glocs
    )
    overflow = max(caps) > CAP_MAX

    if not overflow:
        splits = [np.array_split(glocs[e], N_CORES) for e in range(2)]
        in_maps = []
        for c in range(N_CORES):
            m = dict(sparse_shared)
            for e in range(2):
                cap = caps[e]
                loc = splits[e][c]
                cnt = len(loc)
                xgc = np.zeros((D_MODEL, cap), BF16NP)
                xgc[:, :cnt] = xtb[:, loc]
                # partition-major [p, k, t]
                m[f"xg{e}"] = np.ascontiguousarray(
                    xgc.reshape(KM, P, cap).transpose(1, 0, 2)
                )
                wggc = np.zeros((cap,), np.float32)
                wggc[:cnt] = wge[e][loc]
                m[f"wgg{e}"] = np.ascontiguousarray(
                    np.broadcast_to(wggc, (P, cap))
                )
            in_maps.append(m)

        res = _run(_get_nc("sparse", caps), in_maps)

        y = np.zeros((N_LOCAL, D_MODEL), np.float32)
        for c in range(N_CORES):
            for e in range(2):
                loc = splits[e][c]
                cnt = len(loc)
                r = res.results[c][f"yt{e}"]  # (P, M2, cap) bf16
                yd = r.transpose(1, 0, 2).reshape(D_MODEL, caps[e])
                y[loc] += yd.T[:cnt].astype(np.float32)
        return y

    # dense fallback (vanishingly rare: a gather exceeded capacity)
    xt = np.ascontiguousarray(x.T)
    dense_shared = {}
    for e in range(2):
        dense_shared[f"w1_{e}"] = np.ascontiguousarray(
            np.asarray(inputs[f"W1_{e}"], dtype=np.float32)
        )
        dense_shared[f"w2_{e}"] = np.ascontiguousarray(
            np.asarray(inputs[f"W2_{e}"], dtype=np.float32)
        )
        dense_shared[f"b1t_{e}"] = sparse_shared[f"b1t_{e}"]
        dense_shared[f"b2t_{e}"] = sparse_shared[f"b2t_{e}"]
    in_maps = []
    for c in range(N_CORES):
        tok = slice(c * TOKC, (c + 1) * TOKC)
        m = dict(dense_shared)
        m["xt"] = np.ascontiguousarray(xt[:, tok])
        for e in range(2):
            m[f"wg{e}"] = np.ascontiguousarray(
                np.broadcast_to(wge[e][tok], (P, TOKC)).astype(np.float32)
            )
        in_maps.append(m)
    res = _run(_get_nc("dense"), in_maps)
    ytc = np.concatenate([r["yt"] for r in res.results], axis=1)  # (2048, 8192)
    return np.ascontiguousarray(ytc.T)
